# revision 19
# baseline (speedup 1.0000x reference)
"""FP8StaticLinear Trainium2 kernel.

out = requant_fp8(qdq_fp8(x, s_in) @ (w * s_w).T + bias, s_out)

Sharding: data-parallel over tokens (B*S=16384 -> 2048/core on 8 cores).
Device math: fp8e4 DoubleRow matmuls on the PE array. Both operands are
halved on entry so the OCP-e4m3fn grid (max 448) maps onto TRN fp8e4
(max 240) exactly; scales are folded back in the epilogue.
"""

import numpy as np
import ml_dtypes

import concourse.bass as bass
import concourse.mybir as mybir
from concourse.tile import TileContext
from concourse.vector_clock import ScopedClock
from concourse.bass_utils import run_bass_kernel_spmd

FP8 = mybir.dt.float8e4
F32 = mybir.dt.float32
NP_FP8 = ml_dtypes.float8_e4m3  # TRN fp8e4 (max 240, has inf)

N_CORES = 8
P = 128


# ---------------------------------------------------------------------------
# Workaround: this walrus build rejects >1 sem-wait on the Tile tail Drain
# ("Too many sync wait commands"). Split the waits across single-wait drains.
def _drain_and_barrier(self, tick_clock, wait_clock):
    drain_inst = self.nc.sync.drain()
    wait_clock.add_sem_waits(
        drain_inst.ins, ScopedClock({None: tick_clock.global_clock})
    )
    w = list(drain_inst.ins.sync_info.on_wait)
    if len(w) > 1:
        drain_inst.ins.sync_info = mybir.SyncInfo(on_wait=[w[0]], on_update=[])
        for extra in w[1:]:
            d2 = self.nc.sync.drain()
            d2.ins.sync_info = mybir.SyncInfo(on_wait=[extra], on_update=[])
    self.nc.all_engine_barrier()
    assert self.sems is not None
    popped = self.nc._tile_sem_poison_stack.pop()
    assert popped is self._sem_poison
    self.nc.clear_and_free_semaphores(list(self.sems.allocated().values()))
    self.nc.all_engine_barrier()


TileContext._drain_and_barrier = _drain_and_barrier


def split_sync_waits(nc, max_waits=1):
    """Hoist extra sem-waits onto standalone EventSemaphore carriers.

    This walrus build's setupSyncWait rejects instructions carrying more
    than one sem-wait ("Too many sync wait commands"), so any instruction
    with N>1 waits becomes N-1 single-wait EventSemaphore instructions on
    the same engine followed by the original instruction with one wait.
    """
    n_new = 0
    for f in nc.m.functions:
        for blk in f.blocks:
            out = []
            changed = False
            for inst in blk.instructions:
                si = inst.sync_info
                w = list(si.on_wait) if si is not None else []
                if len(w) > max_waits:
                    upd = list(inst.sync_info.on_update)
                    for wi in w[max_waits:]:
                        es = mybir.InstEventSemaphore(
                            name=f"hoistw-{n_new}", ins=[], outs=[]
                        )
                        n_new += 1
                        es.engine = inst.engine
                        es.sync_info = mybir.SyncInfo(on_wait=[wi], on_update=[])
                        out.append(es)
                    inst.sync_info = mybir.SyncInfo(
                        on_wait=w[:max_waits], on_update=upd
                    )
                    changed = True
                out.append(inst)
            if changed:
                blk.instructions = out
    return nc
# ---------------------------------------------------------------------------


USE_DOUBLE_ROW = True  # fp8 DoubleRow: ~1.5x matmul rate, ~1e-4 accum noise


def build(K, M, N, MF=512, use_dr=None):
    """One-core program: out_t[N, M] = requantized (x @ w.T + b) transposed.

    DRAM inputs:
      xt     [K, M]  f32    x shard, transposed (k-major)
      wt     [N//128, 128, K//128, 128]  fp8   halved weight, tiled
                     wt[nt, p, j, n] = fp8(w[nt*128+n, j*128+p] / 2)
      bias2  [N]     f32    bias / (2*s_out)
      inv2si, alpha, two_os  [1, 1] f32:
                     1/(2*s_in),  2*s_in*s_w/s_out,  2*s_out
    Output:
      out_t  [N, M]  f32
    """
    if use_dr is None:
        use_dr = USE_DOUBLE_ROW
    KS = K // P          # k subtiles of 128
    JP = KS // 2         # DoubleRow pairs
    NT = N // P          # n tiles
    MB = M // MF         # m blocks
    AF = mybir.ActivationFunctionType
    OP = mybir.AluOpType

    nc = bass.Bass()
    xt = nc.dram_tensor("xt", [K, M], F32, kind="ExternalInput")
    wt = nc.dram_tensor("wt", [NT, P, KS, P], FP8, kind="ExternalInput")
    bias2_d = nc.dram_tensor("bias2", [N], F32, kind="ExternalInput")
    inv2si_d = nc.dram_tensor("inv2si", [1, 1], F32, kind="ExternalInput")
    alpha_d = nc.dram_tensor("alpha", [1, 1], F32, kind="ExternalInput")
    two_os_d = nc.dram_tensor("two_os", [1, 1], F32, kind="ExternalInput")
    out_t = nc.dram_tensor("out_t", [N, M], F32, kind="ExternalOutput")

    with TileContext(nc) as tc:
        with (
            tc.tile_pool(name="consts", bufs=1) as consts,
            tc.tile_pool(name="wres", bufs=1) as wres,
            tc.tile_pool(name="qx", bufs=2) as qxp,
            tc.tile_pool(name="xf", bufs=6) as xfp,
            tc.tile_pool(name="psum", bufs=6, space="PSUM") as psp,
            tc.tile_pool(name="epi", bufs=3) as epi,
            tc.tile_pool(name="q8", bufs=3) as q8p,
            tc.tile_pool(name="yout", bufs=4) as yp,
        ):
            # ---- per-partition broadcast of the scalars ----
            inv2si = consts.tile([P, 1], F32)
            alpha = consts.tile([P, 1], F32)
            two_os = consts.tile([P, 1], F32)
            nc.sync.dma_start(inv2si[:], inv2si_d[0:1, 0:1].to_broadcast((P, 1)))
            nc.sync.dma_start(alpha[:], alpha_d[0:1, 0:1].to_broadcast((P, 1)))
            nc.sync.dma_start(two_os[:], two_os_d[0:1, 0:1].to_broadcast((P, 1)))

            # bias2[p, nt] = bias[nt*128+p] / (2*os)
            bias2 = consts.tile([P, NT], F32)
            nc.sync.dma_start(bias2[:], bias2_d.rearrange("(nt p) -> p nt", p=P))

            # quantize chunk j of block mb: qx[p, j, m] = fp8(clamp(x/(2si)))
            # both ops on DVE so ScalarE stays a dedicated psum evictor
            def emit_quant(mb, j, qx):
                xf = xfp.tile([P, MF], F32, tag="xf", name="xf")
                nc.sync.dma_start(
                    xf[:], xt[j * P : (j + 1) * P, mb * MF : (mb + 1) * MF]
                )
                nc.vector.tensor_scalar(
                    xf[:], xf[:], inv2si[:, 0:1], -224.0, OP.mult, OP.max
                )
                nc.vector.tensor_scalar(
                    qx[:, j, :], xf[:], 224.0, None, OP.min
                )

            # resident halved weight, one tile per nt for per-tile dep
            # tracking; loads are interleaved with the first block's x
            # quantize so the PE can start as soon as w[0] + 2 chunks land
            w_tiles = []

            def emit_wload(nt):
                w_nt = wres.tile([P, KS, P], FP8, tag=f"w{nt}", name=f"w{nt}")
                nc.sync.dma_start(w_nt[:], wt[nt, :, :, :])
                w_tiles.append(w_nt)

            emit_wload(0)
            emit_wload(1)
            qx_tiles = {0: qxp.tile([P, KS, MF], FP8, tag="qx", name="qx0")}
            for j in range(KS):
                emit_quant(0, j, qx_tiles[0])
                for nt in range(2 + j * (NT - 2) // KS,
                                2 + (j + 1) * (NT - 2) // KS):
                    emit_wload(nt)

            # ---- main loop over m blocks ----
            for mb in range(MB):
                qx = qx_tiles[mb]
                if mb + 1 < MB:
                    qx_tiles[mb + 1] = qxp.tile([P, KS, MF], FP8, tag="qx", name=f"qx{mb+1}")

                for nt in range(NT):
                    ps = psp.tile([P, MF], F32)
                    if use_dr:
                        for jj in range(JP):
                            nc.tensor.matmul(
                                ps[:],
                                w_tiles[nt][:, 2 * jj : 2 * jj + 2, :],
                                qx[:, 2 * jj : 2 * jj + 2, :],
                                start=(jj == 0),
                                stop=(jj == JP - 1),
                                perf_mode=mybir.MatmulPerfMode.DoubleRow,
                            )
                    else:
                        for j in range(KS):
                            nc.tensor.matmul(
                                ps[:],
                                w_tiles[nt][:, j, :],
                                qx[:, j, :],
                                start=(j == 0),
                                stop=(j == KS - 1),
                            )
                    # epilogue: t = ps*alpha + bias/(2os); q8 = fp8(clamp t);
                    # y = q8 * 2os
                    t = epi.tile([P, MF], F32)
                    nc.scalar.activation(
                        t[:], ps[:], AF.Identity,
                        bias=bias2[:, nt : nt + 1], scale=alpha[:, 0:1],
                    )
                    q8 = q8p.tile([P, MF], FP8)
                    nc.vector.tensor_scalar(
                        q8[:], t[:], -224.0, 224.0, OP.max, OP.min
                    )
                    y = yp.tile([P, MF], F32)
                    nc.vector.tensor_scalar_mul(y[:], q8[:], two_os[:, 0:1])
                    nc.gpsimd.dma_start(
                        out_t[nt * P : (nt + 1) * P, mb * MF : (mb + 1) * MF],
                        y[:],
                    )
                    # interleave next block's quantize so its ACT/DVE work
                    # lands well ahead of this block's end (keeps the PE warm)
                    if mb + 1 < MB and nt < KS:
                        emit_quant(mb + 1, nt, qx_tiles[mb + 1])
    return split_sync_waits(nc)


def prep_weight(weight):
    """[N, K] f32 (e4m3fn-grid values) -> [NT, 128, KS, 128] TRN-fp8 of w/2."""
    N, K = weight.shape
    wq = (weight.astype(np.float32) * 0.5).astype(NP_FP8)
    # [nt, n, j, p] -> [nt, p, j, n]
    return np.ascontiguousarray(
        wq.reshape(N // P, P, K // P, P).transpose(0, 3, 2, 1)
    )


def prep_scalars(weight_scale, bias, input_scale, output_scale):
    si = float(np.asarray(input_scale, np.float64))
    sw = float(np.asarray(weight_scale, np.float64))
    os_ = float(np.asarray(output_scale, np.float64))
    inv2si = np.array([[1.0 / (2.0 * si)]], np.float32)
    alpha = np.array([[2.0 * si * sw / os_]], np.float32)
    two_os = np.array([[2.0 * os_]], np.float32)
    bias2 = (bias.astype(np.float64) / (2.0 * os_)).astype(np.float32)
    return inv2si, alpha, two_os, np.ascontiguousarray(bias2)


def kernel(x, weight, weight_scale, bias, input_scale, output_scale):
    B, S, K = x.shape
    N = weight.shape[0]
    M_total = B * S
    M = M_total // N_CORES

    nc = build(K, M, N)

    xt_full = np.ascontiguousarray(x.reshape(M_total, K).T)  # [K, M_total] f32
    wt = prep_weight(weight)
    inv2si, alpha, two_os, bias2 = prep_scalars(
        weight_scale, bias, input_scale, output_scale
    )

    in_maps = []
    for c in range(N_CORES):
        in_maps.append({
            "xt": np.ascontiguousarray(xt_full[:, c * M : (c + 1) * M]),
            "wt": wt,
            "bias2": bias2,
            "inv2si": inv2si,
            "alpha": alpha,
            "two_os": two_os,
        })

    res = run_bass_kernel_spmd(nc, in_maps, core_ids=list(range(N_CORES)))
    global LAST_RESULT
    LAST_RESULT = res

    out = np.empty((M_total, N), np.float32)
    for c in range(N_CORES):
        out[c * M : (c + 1) * M, :] = res.results[c]["out_t"].T
    return out.reshape(B, S, N)


# revision 20
# speedup vs baseline: 1.0033x; 1.0033x over previous
"""FP8StaticLinear Trainium2 kernel.

out = requant_fp8(qdq_fp8(x, s_in) @ (w * s_w).T + bias, s_out)

Sharding: data-parallel over tokens (B*S=16384 -> 2048/core on 8 cores).
Device math: fp8e4 DoubleRow matmuls on the PE array. Both operands are
halved on entry so the OCP-e4m3fn grid (max 448) maps onto TRN fp8e4
(max 240) exactly; scales are folded back in the epilogue.
"""

import numpy as np
import ml_dtypes

import concourse.bass as bass
import concourse.mybir as mybir
from concourse.tile import TileContext
from concourse.vector_clock import ScopedClock
from concourse.bass_utils import run_bass_kernel_spmd

FP8 = mybir.dt.float8e4
F32 = mybir.dt.float32
NP_FP8 = ml_dtypes.float8_e4m3  # TRN fp8e4 (max 240, has inf)

N_CORES = 8
P = 128


# ---------------------------------------------------------------------------
# Workaround: this walrus build rejects >1 sem-wait on the Tile tail Drain
# ("Too many sync wait commands"). Split the waits across single-wait drains.
def _drain_and_barrier(self, tick_clock, wait_clock):
    drain_inst = self.nc.sync.drain()
    wait_clock.add_sem_waits(
        drain_inst.ins, ScopedClock({None: tick_clock.global_clock})
    )
    w = list(drain_inst.ins.sync_info.on_wait)
    if len(w) > 1:
        drain_inst.ins.sync_info = mybir.SyncInfo(on_wait=[w[0]], on_update=[])
        for extra in w[1:]:
            d2 = self.nc.sync.drain()
            d2.ins.sync_info = mybir.SyncInfo(on_wait=[extra], on_update=[])
    self.nc.all_engine_barrier()
    assert self.sems is not None
    popped = self.nc._tile_sem_poison_stack.pop()
    assert popped is self._sem_poison
    self.nc.clear_and_free_semaphores(list(self.sems.allocated().values()))
    self.nc.all_engine_barrier()


TileContext._drain_and_barrier = _drain_and_barrier


def split_sync_waits(nc, max_waits=1):
    """Hoist extra sem-waits onto standalone EventSemaphore carriers.

    This walrus build's setupSyncWait rejects instructions carrying more
    than one sem-wait ("Too many sync wait commands"), so any instruction
    with N>1 waits becomes N-1 single-wait EventSemaphore instructions on
    the same engine followed by the original instruction with one wait.
    """
    n_new = 0
    for f in nc.m.functions:
        for blk in f.blocks:
            out = []
            changed = False
            for inst in blk.instructions:
                si = inst.sync_info
                w = list(si.on_wait) if si is not None else []
                if len(w) > max_waits:
                    upd = list(inst.sync_info.on_update)
                    for wi in w[max_waits:]:
                        es = mybir.InstEventSemaphore(
                            name=f"hoistw-{n_new}", ins=[], outs=[]
                        )
                        n_new += 1
                        es.engine = inst.engine
                        es.sync_info = mybir.SyncInfo(on_wait=[wi], on_update=[])
                        out.append(es)
                    inst.sync_info = mybir.SyncInfo(
                        on_wait=w[:max_waits], on_update=upd
                    )
                    changed = True
                out.append(inst)
            if changed:
                blk.instructions = out
    return nc
# ---------------------------------------------------------------------------


USE_DOUBLE_ROW = True  # fp8 DoubleRow: ~1.5x matmul rate, ~1e-4 accum noise


def build(K, M, N, MF=512, use_dr=None):
    """One-core program: out_t[N, M] = requantized (x @ w.T + b) transposed.

    DRAM inputs:
      xt     [K, M]  f32    x shard, transposed (k-major)
      wt     [N//128, 128, K//128, 128]  fp8   halved weight, tiled
                     wt[nt, p, j, n] = fp8(w[nt*128+n, j*128+p] / 2)
      bias2  [N]     f32    bias / (2*s_out)
      inv2si, alpha, two_os  [1, 1] f32:
                     1/(2*s_in),  2*s_in*s_w/s_out,  2*s_out
    Output:
      out_t  [N, M]  f32
    """
    if use_dr is None:
        use_dr = USE_DOUBLE_ROW
    KS = K // P          # k subtiles of 128
    JP = KS // 2         # DoubleRow pairs
    NT = N // P          # n tiles
    MB = M // MF         # m blocks
    AF = mybir.ActivationFunctionType
    OP = mybir.AluOpType

    nc = bass.Bass()
    xt = nc.dram_tensor("xt", [K, M], F32, kind="ExternalInput")
    wt = nc.dram_tensor("wt", [NT, P, KS, P], FP8, kind="ExternalInput")
    bias2_d = nc.dram_tensor("bias2", [N], F32, kind="ExternalInput")
    inv2si_d = nc.dram_tensor("inv2si", [1, 1], F32, kind="ExternalInput")
    alpha_d = nc.dram_tensor("alpha", [1, 1], F32, kind="ExternalInput")
    two_os_d = nc.dram_tensor("two_os", [1, 1], F32, kind="ExternalInput")
    out_t = nc.dram_tensor("out_t", [N, M], F32, kind="ExternalOutput")

    with TileContext(nc) as tc:
        with (
            tc.tile_pool(name="consts", bufs=1) as consts,
            tc.tile_pool(name="wres", bufs=1) as wres,
            tc.tile_pool(name="qx", bufs=2) as qxp,
            tc.tile_pool(name="xf", bufs=6) as xfp,
            tc.tile_pool(name="psum", bufs=6, space="PSUM") as psp,
            tc.tile_pool(name="epi", bufs=3) as epi,
            tc.tile_pool(name="q8", bufs=3) as q8p,
            tc.tile_pool(name="yout", bufs=4) as yp,
        ):
            # ---- per-partition broadcast of the scalars ----
            inv2si = consts.tile([P, 1], F32)
            alpha = consts.tile([P, 1], F32)
            two_os = consts.tile([P, 1], F32)
            nc.sync.dma_start(inv2si[:], inv2si_d[0:1, 0:1].to_broadcast((P, 1)))
            nc.sync.dma_start(alpha[:], alpha_d[0:1, 0:1].to_broadcast((P, 1)))
            nc.sync.dma_start(two_os[:], two_os_d[0:1, 0:1].to_broadcast((P, 1)))

            # bias2[p, nt] = bias[nt*128+p] / (2*os)
            bias2 = consts.tile([P, NT], F32)
            nc.sync.dma_start(bias2[:], bias2_d.rearrange("(nt p) -> p nt", p=P))

            # quantize chunk j of block mb: qx[p, j, m] = fp8(x/(2si)).
            # No explicit clamp: |x/(2si)| <= 224*(1+2^-23) by construction
            # of input_scale, and fp8 RNE rounds anything < 232 down to 224,
            # so the mult+cast is exact clamp semantics. Single DVE op keeps
            # ScalarE as a dedicated psum evictor.
            def emit_quant(mb, j, qx):
                xf = xfp.tile([P, MF], F32, tag="xf", name="xf")
                nc.sync.dma_start(
                    xf[:], xt[j * P : (j + 1) * P, mb * MF : (mb + 1) * MF]
                )
                nc.vector.tensor_scalar(
                    qx[:, j, :], xf[:], inv2si[:, 0:1], None, OP.mult
                )

            # resident halved weight, one tile per nt for per-tile dep
            # tracking; loads are interleaved with the first block's x
            # quantize so the PE can start as soon as w[0] + 2 chunks land
            w_tiles = []

            def emit_wload(nt):
                w_nt = wres.tile([P, KS, P], FP8, tag=f"w{nt}", name=f"w{nt}")
                nc.sync.dma_start(w_nt[:], wt[nt, :, :, :])
                w_tiles.append(w_nt)

            emit_wload(0)
            emit_wload(1)
            qx_tiles = {0: qxp.tile([P, KS, MF], FP8, tag="qx", name="qx0")}
            for j in range(KS):
                emit_quant(0, j, qx_tiles[0])
                for nt in range(2 + j * (NT - 2) // KS,
                                2 + (j + 1) * (NT - 2) // KS):
                    emit_wload(nt)

            # ---- main loop over m blocks ----
            for mb in range(MB):
                qx = qx_tiles[mb]
                if mb + 1 < MB:
                    qx_tiles[mb + 1] = qxp.tile([P, KS, MF], FP8, tag="qx", name=f"qx{mb+1}")

                for nt in range(NT):
                    ps = psp.tile([P, MF], F32)
                    if use_dr:
                        for jj in range(JP):
                            nc.tensor.matmul(
                                ps[:],
                                w_tiles[nt][:, 2 * jj : 2 * jj + 2, :],
                                qx[:, 2 * jj : 2 * jj + 2, :],
                                start=(jj == 0),
                                stop=(jj == JP - 1),
                                perf_mode=mybir.MatmulPerfMode.DoubleRow,
                            )
                    else:
                        for j in range(KS):
                            nc.tensor.matmul(
                                ps[:],
                                w_tiles[nt][:, j, :],
                                qx[:, j, :],
                                start=(j == 0),
                                stop=(j == KS - 1),
                            )
                    # epilogue: t = ps*alpha + bias/(2os); q8 = fp8(clamp t);
                    # y = q8 * 2os
                    t = epi.tile([P, MF], F32)
                    nc.scalar.activation(
                        t[:], ps[:], AF.Identity,
                        bias=bias2[:, nt : nt + 1], scale=alpha[:, 0:1],
                    )
                    q8 = q8p.tile([P, MF], FP8)
                    nc.vector.tensor_scalar(
                        q8[:], t[:], -224.0, 224.0, OP.max, OP.min
                    )
                    y = yp.tile([P, MF], F32)
                    nc.vector.tensor_scalar_mul(y[:], q8[:], two_os[:, 0:1])
                    nc.gpsimd.dma_start(
                        out_t[nt * P : (nt + 1) * P, mb * MF : (mb + 1) * MF],
                        y[:],
                    )
                    # interleave next block's quantize so its ACT/DVE work
                    # lands well ahead of this block's end (keeps the PE warm)
                    if mb + 1 < MB and nt < KS:
                        emit_quant(mb + 1, nt, qx_tiles[mb + 1])
    return split_sync_waits(nc)


def prep_weight(weight):
    """[N, K] f32 (e4m3fn-grid values) -> [NT, 128, KS, 128] TRN-fp8 of w/2."""
    N, K = weight.shape
    wq = (weight.astype(np.float32) * 0.5).astype(NP_FP8)
    # [nt, n, j, p] -> [nt, p, j, n]
    return np.ascontiguousarray(
        wq.reshape(N // P, P, K // P, P).transpose(0, 3, 2, 1)
    )


def prep_scalars(weight_scale, bias, input_scale, output_scale):
    si = float(np.asarray(input_scale, np.float64))
    sw = float(np.asarray(weight_scale, np.float64))
    os_ = float(np.asarray(output_scale, np.float64))
    inv2si = np.array([[1.0 / (2.0 * si)]], np.float32)
    alpha = np.array([[2.0 * si * sw / os_]], np.float32)
    two_os = np.array([[2.0 * os_]], np.float32)
    bias2 = (bias.astype(np.float64) / (2.0 * os_)).astype(np.float32)
    return inv2si, alpha, two_os, np.ascontiguousarray(bias2)


def kernel(x, weight, weight_scale, bias, input_scale, output_scale):
    B, S, K = x.shape
    N = weight.shape[0]
    M_total = B * S
    M = M_total // N_CORES

    nc = build(K, M, N)

    xt_full = np.ascontiguousarray(x.reshape(M_total, K).T)  # [K, M_total] f32
    wt = prep_weight(weight)
    inv2si, alpha, two_os, bias2 = prep_scalars(
        weight_scale, bias, input_scale, output_scale
    )

    in_maps = []
    for c in range(N_CORES):
        in_maps.append({
            "xt": np.ascontiguousarray(xt_full[:, c * M : (c + 1) * M]),
            "wt": wt,
            "bias2": bias2,
            "inv2si": inv2si,
            "alpha": alpha,
            "two_os": two_os,
        })

    res = run_bass_kernel_spmd(nc, in_maps, core_ids=list(range(N_CORES)))
    global LAST_RESULT
    LAST_RESULT = res

    out = np.empty((M_total, N), np.float32)
    for c in range(N_CORES):
        out[c * M : (c + 1) * M, :] = res.results[c]["out_t"].T
    return out.reshape(B, S, N)


# revision 23
# speedup vs baseline: 1.0446x; 1.0411x over previous
"""FP8StaticLinear Trainium2 kernel.

out = requant_fp8(qdq_fp8(x, s_in) @ (w * s_w).T + bias, s_out)

Sharding: data-parallel over tokens (B*S=16384 -> 2048/core on 8 cores).
Device math: fp8e4 DoubleRow matmuls on the PE array. Both operands are
halved on entry so the OCP-e4m3fn grid (max 448) maps onto TRN fp8e4
(max 240) exactly; scales are folded back in the epilogue.
"""

import numpy as np
import ml_dtypes

import concourse.bass as bass
import concourse.mybir as mybir
from concourse.tile import TileContext
from concourse.vector_clock import ScopedClock
from concourse.bass_utils import run_bass_kernel_spmd

FP8 = mybir.dt.float8e4
F32 = mybir.dt.float32
NP_FP8 = ml_dtypes.float8_e4m3  # TRN fp8e4 (max 240, has inf)

N_CORES = 8
P = 128


# ---------------------------------------------------------------------------
# Workaround: this walrus build rejects >1 sem-wait on the Tile tail Drain
# ("Too many sync wait commands"). Split the waits across single-wait drains.
def _drain_and_barrier(self, tick_clock, wait_clock):
    drain_inst = self.nc.sync.drain()
    wait_clock.add_sem_waits(
        drain_inst.ins, ScopedClock({None: tick_clock.global_clock})
    )
    w = list(drain_inst.ins.sync_info.on_wait)
    if len(w) > 1:
        drain_inst.ins.sync_info = mybir.SyncInfo(on_wait=[w[0]], on_update=[])
        for extra in w[1:]:
            d2 = self.nc.sync.drain()
            d2.ins.sync_info = mybir.SyncInfo(on_wait=[extra], on_update=[])
    self.nc.all_engine_barrier()
    assert self.sems is not None
    popped = self.nc._tile_sem_poison_stack.pop()
    assert popped is self._sem_poison
    self.nc.clear_and_free_semaphores(list(self.sems.allocated().values()))
    self.nc.all_engine_barrier()


TileContext._drain_and_barrier = _drain_and_barrier


def split_sync_waits(nc, max_waits=1):
    """Hoist extra sem-waits onto standalone EventSemaphore carriers.

    This walrus build's setupSyncWait rejects instructions carrying more
    than one sem-wait ("Too many sync wait commands"), so any instruction
    with N>1 waits becomes N-1 single-wait EventSemaphore instructions on
    the same engine followed by the original instruction with one wait.
    """
    n_new = 0
    for f in nc.m.functions:
        for blk in f.blocks:
            out = []
            changed = False
            for inst in blk.instructions:
                si = inst.sync_info
                w = list(si.on_wait) if si is not None else []
                if len(w) > max_waits:
                    upd = list(inst.sync_info.on_update)
                    for wi in w[max_waits:]:
                        es = mybir.InstEventSemaphore(
                            name=f"hoistw-{n_new}", ins=[], outs=[]
                        )
                        n_new += 1
                        es.engine = inst.engine
                        es.sync_info = mybir.SyncInfo(on_wait=[wi], on_update=[])
                        out.append(es)
                    inst.sync_info = mybir.SyncInfo(
                        on_wait=w[:max_waits], on_update=upd
                    )
                    changed = True
                out.append(inst)
            if changed:
                blk.instructions = out
    return nc
# ---------------------------------------------------------------------------


USE_DOUBLE_ROW = True  # fp8 DoubleRow: ~1.5x matmul rate, ~1e-4 accum noise


def build(K, M, N, MF=512, use_dr=None):
    """One-core program: out_t[N, M] = requantized (x @ w.T + b) transposed.

    DRAM inputs:
      xt     [K, M]  f32    x shard, transposed (k-major)
      wt     [N//128, 128, K//128, 128]  fp8   halved weight, tiled
                     wt[nt, p, j, n] = fp8(w[nt*128+n, j*128+p] / 2)
      bias2  [N]     f32    bias / (2*s_out)
      inv2si, alpha, two_os  [1, 1] f32:
                     1/(2*s_in),  2*s_in*s_w/s_out,  2*s_out
    Output:
      out_t  [N, M]  f32
    """
    if use_dr is None:
        use_dr = USE_DOUBLE_ROW
    KS = K // P          # k subtiles of 128
    JP = KS // 2         # DoubleRow pairs
    NT = N // P          # n tiles
    MB = M // MF         # m blocks
    AF = mybir.ActivationFunctionType
    OP = mybir.AluOpType

    nc = bass.Bass()
    xt = nc.dram_tensor("xt", [K, M], F32, kind="ExternalInput")
    wt = nc.dram_tensor("wt", [NT, P, KS, P], FP8, kind="ExternalInput")
    bias2_d = nc.dram_tensor("bias2", [N], F32, kind="ExternalInput")
    inv2si_d = nc.dram_tensor("inv2si", [1, 1], F32, kind="ExternalInput")
    alpha_d = nc.dram_tensor("alpha", [1, 1], F32, kind="ExternalInput")
    two_os_d = nc.dram_tensor("two_os", [1, 1], F32, kind="ExternalInput")
    out_t = nc.dram_tensor("out_t", [N, M], F32, kind="ExternalOutput")

    with TileContext(nc) as tc:
        with (
            tc.tile_pool(name="consts", bufs=1) as consts,
            tc.tile_pool(name="wres", bufs=1) as wres,
            tc.tile_pool(name="qx", bufs=2) as qxp,
            tc.tile_pool(name="xf", bufs=6) as xfp,
            tc.tile_pool(name="psum", bufs=6, space="PSUM") as psp,
            tc.tile_pool(name="epi", bufs=3) as epi,
            tc.tile_pool(name="q8", bufs=3) as q8p,
            tc.tile_pool(name="yout", bufs=4) as yp,
        ):
            # ---- per-partition broadcast of the scalars ----
            inv2si = consts.tile([P, 1], F32)
            alpha = consts.tile([P, 1], F32)
            two_os = consts.tile([P, 1], F32)
            nc.gpsimd.dma_start(inv2si[:], inv2si_d[0:1, 0:1].to_broadcast((P, 1)))
            nc.gpsimd.dma_start(alpha[:], alpha_d[0:1, 0:1].to_broadcast((P, 1)))
            nc.gpsimd.dma_start(two_os[:], two_os_d[0:1, 0:1].to_broadcast((P, 1)))

            # bias2[p, nt] = bias[nt*128+p] / (2*os)
            bias2 = consts.tile([P, NT], F32)
            nc.gpsimd.dma_start(bias2[:], bias2_d.rearrange("(nt p) -> p nt", p=P))

            # quantize chunk j of block mb: qx[p, j, m] = fp8(x/(2si)).
            # No explicit clamp: |x/(2si)| <= 224*(1+2^-23) by construction
            # of input_scale, and fp8 RNE rounds anything < 232 down to 224,
            # so the mult+cast is exact clamp semantics. Single DVE op keeps
            # ScalarE as a dedicated psum evictor.
            def emit_quant(mb, j, qx):
                xf = xfp.tile([P, MF], F32, tag="xf", name="xf")
                nc.sync.dma_start(
                    xf[:], xt[j * P : (j + 1) * P, mb * MF : (mb + 1) * MF]
                )
                nc.vector.tensor_scalar(
                    qx[:, j, :], xf[:], inv2si[:, 0:1], None, OP.mult
                )

            # resident halved weight, one tile per nt for per-tile dep
            # tracking; loads are interleaved with the first block's x
            # quantize so the PE can start as soon as w[0] + 2 chunks land
            w_tiles = []

            def emit_wload(nt):
                w_nt = wres.tile([P, KS, P], FP8, tag=f"w{nt}", name=f"w{nt}")
                nc.sync.dma_start(w_nt[:], wt[nt, :, :, :])
                w_tiles.append(w_nt)

            # x chunks get queue priority (the PE is gated on the full qx0);
            # weight tiles trail behind -- they are consumed at ~1/3 the
            # delivery rate, so w[nt] stays ahead of group nt anyway
            qx_tiles = {0: qxp.tile([P, KS, MF], FP8, tag="qx", name="qx0")}
            emit_quant(0, 0, qx_tiles[0])
            emit_quant(0, 1, qx_tiles[0])
            emit_wload(0)
            for j in range(2, KS):
                emit_quant(0, j, qx_tiles[0])
            for nt in range(1, NT):
                emit_wload(nt)

            # ---- main loop over m blocks ----
            for mb in range(MB):
                qx = qx_tiles[mb]
                if mb + 1 < MB:
                    qx_tiles[mb + 1] = qxp.tile([P, KS, MF], FP8, tag="qx", name=f"qx{mb+1}")

                for nt in range(NT):
                    ps = psp.tile([P, MF], F32)
                    if use_dr:
                        for jj in range(JP):
                            nc.tensor.matmul(
                                ps[:],
                                w_tiles[nt][:, 2 * jj : 2 * jj + 2, :],
                                qx[:, 2 * jj : 2 * jj + 2, :],
                                start=(jj == 0),
                                stop=(jj == JP - 1),
                                perf_mode=mybir.MatmulPerfMode.DoubleRow,
                            )
                    else:
                        for j in range(KS):
                            nc.tensor.matmul(
                                ps[:],
                                w_tiles[nt][:, j, :],
                                qx[:, j, :],
                                start=(j == 0),
                                stop=(j == KS - 1),
                            )
                    # epilogue: t = ps*alpha + bias/(2os); q8 = fp8(clamp t);
                    # y = q8 * 2os
                    t = epi.tile([P, MF], F32)
                    nc.scalar.activation(
                        t[:], ps[:], AF.Identity,
                        bias=bias2[:, nt : nt + 1], scale=alpha[:, 0:1],
                    )
                    q8 = q8p.tile([P, MF], FP8)
                    nc.vector.tensor_scalar(
                        q8[:], t[:], -224.0, 224.0, OP.max, OP.min
                    )
                    y = yp.tile([P, MF], F32)
                    nc.vector.tensor_scalar_mul(y[:], q8[:], two_os[:, 0:1])
                    nc.gpsimd.dma_start(
                        out_t[nt * P : (nt + 1) * P, mb * MF : (mb + 1) * MF],
                        y[:],
                    )
                    # interleave next block's quantize so its ACT/DVE work
                    # lands well ahead of this block's end (keeps the PE warm)
                    if mb + 1 < MB and nt < KS:
                        emit_quant(mb + 1, nt, qx_tiles[mb + 1])
    return split_sync_waits(nc)


def prep_weight(weight):
    """[N, K] f32 (e4m3fn-grid values) -> [NT, 128, KS, 128] TRN-fp8 of w/2."""
    N, K = weight.shape
    wq = (weight.astype(np.float32) * 0.5).astype(NP_FP8)
    # [nt, n, j, p] -> [nt, p, j, n]
    return np.ascontiguousarray(
        wq.reshape(N // P, P, K // P, P).transpose(0, 3, 2, 1)
    )


def prep_scalars(weight_scale, bias, input_scale, output_scale):
    si = float(np.asarray(input_scale, np.float64))
    sw = float(np.asarray(weight_scale, np.float64))
    os_ = float(np.asarray(output_scale, np.float64))
    inv2si = np.array([[1.0 / (2.0 * si)]], np.float32)
    alpha = np.array([[2.0 * si * sw / os_]], np.float32)
    two_os = np.array([[2.0 * os_]], np.float32)
    bias2 = (bias.astype(np.float64) / (2.0 * os_)).astype(np.float32)
    return inv2si, alpha, two_os, np.ascontiguousarray(bias2)


def kernel(x, weight, weight_scale, bias, input_scale, output_scale):
    B, S, K = x.shape
    N = weight.shape[0]
    M_total = B * S
    M = M_total // N_CORES

    nc = build(K, M, N)

    xt_full = np.ascontiguousarray(x.reshape(M_total, K).T)  # [K, M_total] f32
    wt = prep_weight(weight)
    inv2si, alpha, two_os, bias2 = prep_scalars(
        weight_scale, bias, input_scale, output_scale
    )

    in_maps = []
    for c in range(N_CORES):
        in_maps.append({
            "xt": np.ascontiguousarray(xt_full[:, c * M : (c + 1) * M]),
            "wt": wt,
            "bias2": bias2,
            "inv2si": inv2si,
            "alpha": alpha,
            "two_os": two_os,
        })

    res = run_bass_kernel_spmd(nc, in_maps, core_ids=list(range(N_CORES)))
    global LAST_RESULT
    LAST_RESULT = res

    out = np.empty((M_total, N), np.float32)
    for c in range(N_CORES):
        out[c * M : (c + 1) * M, :] = res.results[c]["out_t"].T
    return out.reshape(B, S, N)


# revision 25
# speedup vs baseline: 1.0455x; 1.0009x over previous
"""FP8StaticLinear Trainium2 kernel.

out = requant_fp8(qdq_fp8(x, s_in) @ (w * s_w).T + bias, s_out)

Sharding: data-parallel over tokens (B*S=16384 -> 2048/core on 8 cores).
Device math: fp8e4 DoubleRow matmuls on the PE array. Both operands are
halved on entry so the OCP-e4m3fn grid (max 448) maps onto TRN fp8e4
(max 240) exactly; scales are folded back in the epilogue.
"""

import numpy as np
import ml_dtypes

import concourse.bass as bass
import concourse.mybir as mybir
from concourse.tile import TileContext
from concourse.vector_clock import ScopedClock
from concourse.bass_utils import run_bass_kernel_spmd

FP8 = mybir.dt.float8e4
F32 = mybir.dt.float32
NP_FP8 = ml_dtypes.float8_e4m3  # TRN fp8e4 (max 240, has inf)

N_CORES = 8
P = 128


# ---------------------------------------------------------------------------
# Workaround: this walrus build rejects >1 sem-wait on the Tile tail Drain
# ("Too many sync wait commands"). Split the waits across single-wait drains.
def _drain_and_barrier(self, tick_clock, wait_clock):
    drain_inst = self.nc.sync.drain()
    wait_clock.add_sem_waits(
        drain_inst.ins, ScopedClock({None: tick_clock.global_clock})
    )
    w = list(drain_inst.ins.sync_info.on_wait)
    if len(w) > 1:
        drain_inst.ins.sync_info = mybir.SyncInfo(on_wait=[w[0]], on_update=[])
        for extra in w[1:]:
            d2 = self.nc.sync.drain()
            d2.ins.sync_info = mybir.SyncInfo(on_wait=[extra], on_update=[])
    self.nc.all_engine_barrier()
    assert self.sems is not None
    popped = self.nc._tile_sem_poison_stack.pop()
    assert popped is self._sem_poison
    self.nc.clear_and_free_semaphores(list(self.sems.allocated().values()))
    self.nc.all_engine_barrier()


TileContext._drain_and_barrier = _drain_and_barrier


def split_sync_waits(nc, max_waits=1):
    """Hoist extra sem-waits onto standalone EventSemaphore carriers.

    This walrus build's setupSyncWait rejects instructions carrying more
    than one sem-wait ("Too many sync wait commands"), so any instruction
    with N>1 waits becomes N-1 single-wait EventSemaphore instructions on
    the same engine followed by the original instruction with one wait.
    """
    n_new = 0
    for f in nc.m.functions:
        for blk in f.blocks:
            out = []
            changed = False
            for inst in blk.instructions:
                si = inst.sync_info
                w = list(si.on_wait) if si is not None else []
                if len(w) > max_waits:
                    upd = list(inst.sync_info.on_update)
                    for wi in w[max_waits:]:
                        es = mybir.InstEventSemaphore(
                            name=f"hoistw-{n_new}", ins=[], outs=[]
                        )
                        n_new += 1
                        es.engine = inst.engine
                        es.sync_info = mybir.SyncInfo(on_wait=[wi], on_update=[])
                        out.append(es)
                    inst.sync_info = mybir.SyncInfo(
                        on_wait=w[:max_waits], on_update=upd
                    )
                    changed = True
                out.append(inst)
            if changed:
                blk.instructions = out
    return nc
# ---------------------------------------------------------------------------


USE_DOUBLE_ROW = True  # fp8 DoubleRow: ~1.5x matmul rate, ~1e-4 accum noise


def build(K, M, N, MF=512, use_dr=None):
    """One-core program: out_t[N, M] = requantized (x @ w.T + b) transposed.

    DRAM inputs:
      xt     [K, M]  f32    x shard, transposed (k-major)
      wt     [N//128, 128, K//128, 128]  fp8   halved weight, tiled
                     wt[nt, p, j, n] = fp8(w[nt*128+n, j*128+p] / 2)
      bias2  [N]     f32    bias / (2*s_out)
      inv2si, alpha, two_os  [1, 1] f32:
                     1/(2*s_in),  2*s_in*s_w/s_out,  2*s_out
    Output:
      out_t  [N, M]  f32
    """
    if use_dr is None:
        use_dr = USE_DOUBLE_ROW
    KS = K // P          # k subtiles of 128
    JP = KS // 2         # DoubleRow pairs
    NT = N // P          # n tiles
    MB = M // MF         # m blocks
    AF = mybir.ActivationFunctionType
    OP = mybir.AluOpType

    nc = bass.Bass()
    xt = nc.dram_tensor("xt", [K, M], F32, kind="ExternalInput")
    wt = nc.dram_tensor("wt", [NT, P, KS, P], FP8, kind="ExternalInput")
    bias2_d = nc.dram_tensor("bias2", [N], F32, kind="ExternalInput")
    inv2si_d = nc.dram_tensor("inv2si", [1, 1], F32, kind="ExternalInput")
    alpha_d = nc.dram_tensor("alpha", [1, 1], F32, kind="ExternalInput")
    two_os_d = nc.dram_tensor("two_os", [1, 1], F32, kind="ExternalInput")
    out_t = nc.dram_tensor("out_t", [N, M], F32, kind="ExternalOutput")

    with TileContext(nc) as tc:
        with (
            tc.tile_pool(name="consts", bufs=1) as consts,
            tc.tile_pool(name="wres", bufs=1) as wres,
            tc.tile_pool(name="qx", bufs=2) as qxp,
            tc.tile_pool(name="xf", bufs=6) as xfp,
            tc.tile_pool(name="psum", bufs=6, space="PSUM") as psp,
            tc.tile_pool(name="epi", bufs=3) as epi,
            tc.tile_pool(name="q8", bufs=3) as q8p,
            tc.tile_pool(name="yout", bufs=4) as yp,
        ):
            # ---- per-partition broadcast of the scalars ----
            inv2si = consts.tile([P, 1], F32)
            alpha = consts.tile([P, 1], F32)
            two_os = consts.tile([P, 1], F32)
            nc.gpsimd.dma_start(inv2si[:], inv2si_d[0:1, 0:1].to_broadcast((P, 1)))
            nc.gpsimd.dma_start(alpha[:], alpha_d[0:1, 0:1].to_broadcast((P, 1)))
            nc.gpsimd.dma_start(two_os[:], two_os_d[0:1, 0:1].to_broadcast((P, 1)))

            # bias2[p, nt] = bias[nt*128+p] / (2*os)
            bias2 = consts.tile([P, NT], F32)
            nc.gpsimd.dma_start(bias2[:], bias2_d.rearrange("(nt p) -> p nt", p=P))

            # quantize chunk j of block mb: qx[p, j, m] = fp8(x/(2si)).
            # No explicit clamp: |x/(2si)| <= 224*(1+2^-23) by construction
            # of input_scale, and fp8 RNE rounds anything < 232 down to 224,
            # so the mult+cast is exact clamp semantics. Single DVE op keeps
            # ScalarE as a dedicated psum evictor.
            def emit_quant(mb, j, qx):
                xf = xfp.tile([P, MF], F32, tag="xf", name="xf")
                nc.sync.dma_start(
                    xf[:], xt[j * P : (j + 1) * P, mb * MF : (mb + 1) * MF]
                )
                nc.vector.tensor_scalar(
                    qx[:, j, :], xf[:], inv2si[:, 0:1], None, OP.mult
                )

            # resident halved weight, one tile per nt for per-tile dep
            # tracking; loads are interleaved with the first block's x
            # quantize so the PE can start as soon as w[0] + 2 chunks land
            w_tiles = []

            def emit_wload(nt):
                w_nt = wres.tile([P, KS, P], FP8, tag=f"w{nt}", name=f"w{nt}")
                nc.sync.dma_start(w_nt[:], wt[nt, :, :, :])
                w_tiles.append(w_nt)

            # x chunks get queue priority (the PE is gated on the full qx0);
            # only the warm-up groups' weights are hoisted between early
            # chunks, the rest trail (consumed at ~1/3 the delivery rate)
            NW = min(6, NT)
            qx_tiles = {0: qxp.tile([P, KS, MF], FP8, tag="qx", name="qx0")}
            emit_quant(0, 0, qx_tiles[0])
            emit_quant(0, 1, qx_tiles[0])
            emit_wload(0)
            emit_wload(1)
            for j in range(2, KS):
                emit_quant(0, j, qx_tiles[0])
                if j < NW:
                    emit_wload(j)
            for nt in range(NW, NT):
                emit_wload(nt)

            def emit_mms(ps, nt, qx):
                if use_dr:
                    for jj in range(JP):
                        nc.tensor.matmul(
                            ps[:],
                            w_tiles[nt][:, 2 * jj : 2 * jj + 2, :],
                            qx[:, 2 * jj : 2 * jj + 2, :],
                            start=(jj == 0),
                            stop=(jj == JP - 1),
                            perf_mode=mybir.MatmulPerfMode.DoubleRow,
                        )
                else:
                    for j in range(KS):
                        nc.tensor.matmul(
                            ps[:],
                            w_tiles[nt][:, j, :],
                            qx[:, j, :],
                            start=(j == 0),
                            stop=(j == KS - 1),
                        )

            # epilogue: t = ps*alpha + bias/(2os); q8 = fp8(clamp t);
            # y = q8 * 2os
            def emit_epilogue(ps, nt, mb):
                t = epi.tile([P, MF], F32, tag="t", name="t")
                nc.scalar.activation(
                    t[:], ps[:], AF.Identity,
                    bias=bias2[:, nt : nt + 1], scale=alpha[:, 0:1],
                )
                q8 = q8p.tile([P, MF], FP8, tag="q8", name="q8")
                nc.vector.tensor_scalar(
                    q8[:], t[:], -224.0, 224.0, OP.max, OP.min
                )
                y = yp.tile([P, MF], F32, tag="y", name="y")
                nc.vector.tensor_scalar_mul(y[:], q8[:], two_os[:, 0:1])
                nc.gpsimd.dma_start(
                    out_t[nt * P : (nt + 1) * P, mb * MF : (mb + 1) * MF],
                    y[:],
                )

            # ---- main loop over m blocks ----
            for mb in range(MB):
                qx = qx_tiles[mb]
                if mb + 1 < MB:
                    qx_tiles[mb + 1] = qxp.tile(
                        [P, KS, MF], FP8, tag="qx", name=f"qx{mb+1}"
                    )

                if mb == 0 and use_dr:
                    # warm-up: first NW groups accumulate k-outer across NW
                    # psum banks, so the PE issues NW matmuls per arriving
                    # chunk pair instead of idling for the full qx0
                    ps_warm = [
                        psp.tile([P, MF], F32, tag="ps", name=f"psw{g}")
                        for g in range(NW)
                    ]
                    for jj in range(JP):
                        for g in range(NW):
                            nc.tensor.matmul(
                                ps_warm[g][:],
                                w_tiles[g][:, 2 * jj : 2 * jj + 2, :],
                                qx[:, 2 * jj : 2 * jj + 2, :],
                                start=(jj == 0),
                                stop=(jj == JP - 1),
                                perf_mode=mybir.MatmulPerfMode.DoubleRow,
                            )
                    for g in range(NW):
                        emit_epilogue(ps_warm[g], g, mb)
                    nt_range = list(range(NW, NT))
                else:
                    nt_range = list(range(NT))

                for idx, nt in enumerate(nt_range):
                    ps = psp.tile([P, MF], F32, tag="ps", name="ps")
                    emit_mms(ps, nt, qx)
                    emit_epilogue(ps, nt, mb)
                    # interleave next block's quantize so its DMA/DVE work
                    # lands well ahead of this block's end (keeps the PE warm)
                    if mb + 1 < MB:
                        lo = idx * KS // len(nt_range)
                        hi = (idx + 1) * KS // len(nt_range)
                        for jq in range(lo, hi):
                            emit_quant(mb + 1, jq, qx_tiles[mb + 1])
    return split_sync_waits(nc)


def prep_weight(weight):
    """[N, K] f32 (e4m3fn-grid values) -> [NT, 128, KS, 128] TRN-fp8 of w/2."""
    N, K = weight.shape
    wq = (weight.astype(np.float32) * 0.5).astype(NP_FP8)
    # [nt, n, j, p] -> [nt, p, j, n]
    return np.ascontiguousarray(
        wq.reshape(N // P, P, K // P, P).transpose(0, 3, 2, 1)
    )


def prep_scalars(weight_scale, bias, input_scale, output_scale):
    si = float(np.asarray(input_scale, np.float64))
    sw = float(np.asarray(weight_scale, np.float64))
    os_ = float(np.asarray(output_scale, np.float64))
    inv2si = np.array([[1.0 / (2.0 * si)]], np.float32)
    alpha = np.array([[2.0 * si * sw / os_]], np.float32)
    two_os = np.array([[2.0 * os_]], np.float32)
    bias2 = (bias.astype(np.float64) / (2.0 * os_)).astype(np.float32)
    return inv2si, alpha, two_os, np.ascontiguousarray(bias2)


def kernel(x, weight, weight_scale, bias, input_scale, output_scale):
    B, S, K = x.shape
    N = weight.shape[0]
    M_total = B * S
    M = M_total // N_CORES

    nc = build(K, M, N)

    xt_full = np.ascontiguousarray(x.reshape(M_total, K).T)  # [K, M_total] f32
    wt = prep_weight(weight)
    inv2si, alpha, two_os, bias2 = prep_scalars(
        weight_scale, bias, input_scale, output_scale
    )

    in_maps = []
    for c in range(N_CORES):
        in_maps.append({
            "xt": np.ascontiguousarray(xt_full[:, c * M : (c + 1) * M]),
            "wt": wt,
            "bias2": bias2,
            "inv2si": inv2si,
            "alpha": alpha,
            "two_os": two_os,
        })

    res = run_bass_kernel_spmd(nc, in_maps, core_ids=list(range(N_CORES)))
    global LAST_RESULT
    LAST_RESULT = res

    out = np.empty((M_total, N), np.float32)
    for c in range(N_CORES):
        out[c * M : (c + 1) * M, :] = res.results[c]["out_t"].T
    return out.reshape(B, S, N)


# revision 26
# speedup vs baseline: 1.0629x; 1.0166x over previous
"""FP8StaticLinear Trainium2 kernel.

out = requant_fp8(qdq_fp8(x, s_in) @ (w * s_w).T + bias, s_out)

Sharding: data-parallel over tokens (B*S=16384 -> 2048/core on 8 cores).
Device math: fp8e4 DoubleRow matmuls on the PE array. Both operands are
halved on entry so the OCP-e4m3fn grid (max 448) maps onto TRN fp8e4
(max 240) exactly; scales are folded back in the epilogue.
"""

import numpy as np
import ml_dtypes

import concourse.bass as bass
import concourse.mybir as mybir
from concourse.tile import TileContext
from concourse.vector_clock import ScopedClock
from concourse.bass_utils import run_bass_kernel_spmd

FP8 = mybir.dt.float8e4
F32 = mybir.dt.float32
NP_FP8 = ml_dtypes.float8_e4m3  # TRN fp8e4 (max 240, has inf)

N_CORES = 8
P = 128


# ---------------------------------------------------------------------------
# Workaround: this walrus build rejects >1 sem-wait on the Tile tail Drain
# ("Too many sync wait commands"). Split the waits across single-wait drains.
def _drain_and_barrier(self, tick_clock, wait_clock):
    drain_inst = self.nc.sync.drain()
    wait_clock.add_sem_waits(
        drain_inst.ins, ScopedClock({None: tick_clock.global_clock})
    )
    w = list(drain_inst.ins.sync_info.on_wait)
    if len(w) > 1:
        drain_inst.ins.sync_info = mybir.SyncInfo(on_wait=[w[0]], on_update=[])
        for extra in w[1:]:
            d2 = self.nc.sync.drain()
            d2.ins.sync_info = mybir.SyncInfo(on_wait=[extra], on_update=[])
    self.nc.all_engine_barrier()
    assert self.sems is not None
    popped = self.nc._tile_sem_poison_stack.pop()
    assert popped is self._sem_poison
    self.nc.clear_and_free_semaphores(list(self.sems.allocated().values()))
    self.nc.all_engine_barrier()


TileContext._drain_and_barrier = _drain_and_barrier


def split_sync_waits(nc, max_waits=1):
    """Hoist extra sem-waits onto standalone EventSemaphore carriers.

    This walrus build's setupSyncWait rejects instructions carrying more
    than one sem-wait ("Too many sync wait commands"), so any instruction
    with N>1 waits becomes N-1 single-wait EventSemaphore instructions on
    the same engine followed by the original instruction with one wait.
    """
    n_new = 0
    for f in nc.m.functions:
        for blk in f.blocks:
            out = []
            changed = False
            for inst in blk.instructions:
                si = inst.sync_info
                w = list(si.on_wait) if si is not None else []
                if len(w) > max_waits:
                    upd = list(inst.sync_info.on_update)
                    for wi in w[max_waits:]:
                        es = mybir.InstEventSemaphore(
                            name=f"hoistw-{n_new}", ins=[], outs=[]
                        )
                        n_new += 1
                        es.engine = inst.engine
                        es.sync_info = mybir.SyncInfo(on_wait=[wi], on_update=[])
                        out.append(es)
                    inst.sync_info = mybir.SyncInfo(
                        on_wait=w[:max_waits], on_update=upd
                    )
                    changed = True
                out.append(inst)
            if changed:
                blk.instructions = out
    return nc
# ---------------------------------------------------------------------------


USE_DOUBLE_ROW = True  # fp8 DoubleRow: ~1.5x matmul rate, ~1e-4 accum noise


def build(K, M, N, MF=512, use_dr=None):
    """One-core program: out_t[N, M] = requantized (x @ w.T + b) transposed.

    DRAM inputs:
      xt     [K, M]  f32    x shard, transposed (k-major)
      wt     [N//128, 128, K//128, 128]  fp8   halved weight, tiled
                     wt[nt, p, j, n] = fp8(w[nt*128+n, j*128+p] / 2)
      bias2  [N]     f32    bias / (2*s_out)
      inv2si, alpha, two_os  [1, 1] f32:
                     1/(2*s_in),  2*s_in*s_w/s_out,  2*s_out
    Output:
      out_t  [N, M]  f32
    """
    if use_dr is None:
        use_dr = USE_DOUBLE_ROW
    KS = K // P          # k subtiles of 128
    JP = KS // 2         # DoubleRow pairs
    NT = N // P          # n tiles
    MB = M // MF         # m blocks
    AF = mybir.ActivationFunctionType
    OP = mybir.AluOpType

    nc = bass.Bass()
    xt = nc.dram_tensor("xt", [K, M], F32, kind="ExternalInput")
    wt = nc.dram_tensor("wt", [NT, P, KS, P], FP8, kind="ExternalInput")
    bias2_d = nc.dram_tensor("bias2", [N], F32, kind="ExternalInput")
    inv2si_d = nc.dram_tensor("inv2si", [1, 1], F32, kind="ExternalInput")
    alpha_d = nc.dram_tensor("alpha", [1, 1], F32, kind="ExternalInput")
    two_os_d = nc.dram_tensor("two_os", [1, 1], F32, kind="ExternalInput")
    out_t = nc.dram_tensor("out_t", [N, M], F32, kind="ExternalOutput")

    with TileContext(nc) as tc:
        with (
            tc.tile_pool(name="consts", bufs=1) as consts,
            tc.tile_pool(name="wres", bufs=1) as wres,
            tc.tile_pool(name="qx", bufs=2) as qxp,
            tc.tile_pool(name="xf", bufs=6) as xfp,
            tc.tile_pool(name="psum", bufs=8, space="PSUM") as psp,
            tc.tile_pool(name="epi", bufs=3) as epi,
            tc.tile_pool(name="q8", bufs=3) as q8p,
            tc.tile_pool(name="yout", bufs=4) as yp,
        ):
            # ---- per-partition broadcast of the scalars ----
            inv2si = consts.tile([P, 1], F32)
            alpha = consts.tile([P, 1], F32)
            two_os = consts.tile([P, 1], F32)
            nc.gpsimd.dma_start(inv2si[:], inv2si_d[0:1, 0:1].to_broadcast((P, 1)))
            nc.gpsimd.dma_start(alpha[:], alpha_d[0:1, 0:1].to_broadcast((P, 1)))
            nc.gpsimd.dma_start(two_os[:], two_os_d[0:1, 0:1].to_broadcast((P, 1)))

            # bias2[p, nt] = bias[nt*128+p] / (2*os)
            bias2 = consts.tile([P, NT], F32)
            nc.gpsimd.dma_start(bias2[:], bias2_d.rearrange("(nt p) -> p nt", p=P))

            # quantize chunk j of block mb: qx[p, j, m] = fp8(x/(2si)).
            # No explicit clamp: |x/(2si)| <= 224*(1+2^-23) by construction
            # of input_scale, and fp8 RNE rounds anything < 232 down to 224,
            # so the mult+cast is exact clamp semantics. Single DVE op keeps
            # ScalarE as a dedicated psum evictor.
            def emit_quant(mb, j, qx):
                xf = xfp.tile([P, MF], F32, tag="xf", name="xf")
                nc.sync.dma_start(
                    xf[:], xt[j * P : (j + 1) * P, mb * MF : (mb + 1) * MF]
                )
                nc.vector.tensor_scalar(
                    qx[:, j, :], xf[:], inv2si[:, 0:1], None, OP.mult
                )

            # resident halved weight, one tile per nt for per-tile dep
            # tracking; loads are interleaved with the first block's x
            # quantize so the PE can start as soon as w[0] + 2 chunks land
            w_tiles = []

            def emit_wload(nt):
                w_nt = wres.tile([P, KS, P], FP8, tag=f"w{nt}", name=f"w{nt}")
                nc.sync.dma_start(w_nt[:], wt[nt, :, :, :])
                w_tiles.append(w_nt)

            # x chunks get queue priority (the PE is gated on the full qx0);
            # only the warm-up groups' weights are hoisted between early
            # chunks, the rest trail (consumed at ~1/3 the delivery rate)
            NW = min(8, NT)
            qx_tiles = {0: qxp.tile([P, KS, MF], FP8, tag="qx", name="qx0")}
            emit_wload(0)
            emit_wload(1)
            emit_quant(0, 0, qx_tiles[0])
            emit_quant(0, 1, qx_tiles[0])
            for j in range(2, KS):
                emit_quant(0, j, qx_tiles[0])
                if j < NW:
                    emit_wload(j)
            for nt in range(NW, NT):
                emit_wload(nt)

            def emit_mms(ps, nt, qx):
                if use_dr:
                    for jj in range(JP):
                        nc.tensor.matmul(
                            ps[:],
                            w_tiles[nt][:, 2 * jj : 2 * jj + 2, :],
                            qx[:, 2 * jj : 2 * jj + 2, :],
                            start=(jj == 0),
                            stop=(jj == JP - 1),
                            perf_mode=mybir.MatmulPerfMode.DoubleRow,
                        )
                else:
                    for j in range(KS):
                        nc.tensor.matmul(
                            ps[:],
                            w_tiles[nt][:, j, :],
                            qx[:, j, :],
                            start=(j == 0),
                            stop=(j == KS - 1),
                        )

            # epilogue: t = ps*alpha + bias/(2os); q8 = fp8(clamp t);
            # y = q8 * 2os
            def emit_epilogue(ps, nt, mb):
                t = epi.tile([P, MF], F32, tag="t", name="t")
                nc.scalar.activation(
                    t[:], ps[:], AF.Identity,
                    bias=bias2[:, nt : nt + 1], scale=alpha[:, 0:1],
                )
                q8 = q8p.tile([P, MF], FP8, tag="q8", name="q8")
                nc.vector.tensor_scalar(
                    q8[:], t[:], -224.0, 224.0, OP.max, OP.min
                )
                y = yp.tile([P, MF], F32, tag="y", name="y")
                nc.vector.tensor_scalar_mul(y[:], q8[:], two_os[:, 0:1])
                nc.gpsimd.dma_start(
                    out_t[nt * P : (nt + 1) * P, mb * MF : (mb + 1) * MF],
                    y[:],
                )

            # ---- main loop over m blocks ----
            for mb in range(MB):
                qx = qx_tiles[mb]
                if mb + 1 < MB:
                    qx_tiles[mb + 1] = qxp.tile(
                        [P, KS, MF], FP8, tag="qx", name=f"qx{mb+1}"
                    )

                if mb == 0 and use_dr:
                    # warm-up: first NW groups accumulate k-outer across NW
                    # psum banks, so the PE issues NW matmuls per arriving
                    # chunk pair instead of idling for the full qx0
                    ps_warm = [
                        psp.tile([P, MF], F32, tag="ps", name=f"psw{g}")
                        for g in range(NW)
                    ]
                    for jj in range(JP):
                        for g in range(NW):
                            nc.tensor.matmul(
                                ps_warm[g][:],
                                w_tiles[g][:, 2 * jj : 2 * jj + 2, :],
                                qx[:, 2 * jj : 2 * jj + 2, :],
                                start=(jj == 0),
                                stop=(jj == JP - 1),
                                perf_mode=mybir.MatmulPerfMode.DoubleRow,
                            )
                    for g in range(NW):
                        emit_epilogue(ps_warm[g], g, mb)
                    nt_range = list(range(NW, NT))
                else:
                    nt_range = list(range(NT))

                for idx, nt in enumerate(nt_range):
                    ps = psp.tile([P, MF], F32, tag="ps", name="ps")
                    emit_mms(ps, nt, qx)
                    emit_epilogue(ps, nt, mb)
                    # interleave next block's quantize so its DMA/DVE work
                    # lands well ahead of this block's end (keeps the PE warm)
                    if mb + 1 < MB:
                        lo = idx * KS // len(nt_range)
                        hi = (idx + 1) * KS // len(nt_range)
                        for jq in range(lo, hi):
                            emit_quant(mb + 1, jq, qx_tiles[mb + 1])
    return split_sync_waits(nc)


def prep_weight(weight):
    """[N, K] f32 (e4m3fn-grid values) -> [NT, 128, KS, 128] TRN-fp8 of w/2."""
    N, K = weight.shape
    wq = (weight.astype(np.float32) * 0.5).astype(NP_FP8)
    # [nt, n, j, p] -> [nt, p, j, n]
    return np.ascontiguousarray(
        wq.reshape(N // P, P, K // P, P).transpose(0, 3, 2, 1)
    )


def prep_scalars(weight_scale, bias, input_scale, output_scale):
    si = float(np.asarray(input_scale, np.float64))
    sw = float(np.asarray(weight_scale, np.float64))
    os_ = float(np.asarray(output_scale, np.float64))
    inv2si = np.array([[1.0 / (2.0 * si)]], np.float32)
    alpha = np.array([[2.0 * si * sw / os_]], np.float32)
    two_os = np.array([[2.0 * os_]], np.float32)
    bias2 = (bias.astype(np.float64) / (2.0 * os_)).astype(np.float32)
    return inv2si, alpha, two_os, np.ascontiguousarray(bias2)


def kernel(x, weight, weight_scale, bias, input_scale, output_scale):
    B, S, K = x.shape
    N = weight.shape[0]
    M_total = B * S
    M = M_total // N_CORES

    nc = build(K, M, N)

    xt_full = np.ascontiguousarray(x.reshape(M_total, K).T)  # [K, M_total] f32
    wt = prep_weight(weight)
    inv2si, alpha, two_os, bias2 = prep_scalars(
        weight_scale, bias, input_scale, output_scale
    )

    in_maps = []
    for c in range(N_CORES):
        in_maps.append({
            "xt": np.ascontiguousarray(xt_full[:, c * M : (c + 1) * M]),
            "wt": wt,
            "bias2": bias2,
            "inv2si": inv2si,
            "alpha": alpha,
            "two_os": two_os,
        })

    res = run_bass_kernel_spmd(nc, in_maps, core_ids=list(range(N_CORES)))
    global LAST_RESULT
    LAST_RESULT = res

    out = np.empty((M_total, N), np.float32)
    for c in range(N_CORES):
        out[c * M : (c + 1) * M, :] = res.results[c]["out_t"].T
    return out.reshape(B, S, N)


# revision 28
# speedup vs baseline: 1.0815x; 1.0175x over previous
"""FP8StaticLinear Trainium2 kernel.

out = requant_fp8(qdq_fp8(x, s_in) @ (w * s_w).T + bias, s_out)

Sharding: data-parallel over tokens (B*S=16384 -> 2048/core on 8 cores).
Device math: fp8e4 DoubleRow matmuls on the PE array. Both operands are
halved on entry so the OCP-e4m3fn grid (max 448) maps onto TRN fp8e4
(max 240) exactly; scales are folded back in the epilogue.
"""

import numpy as np
import ml_dtypes

import concourse.bass as bass
import concourse.mybir as mybir
from concourse.tile import TileContext
from concourse.vector_clock import ScopedClock
from concourse.bass_utils import run_bass_kernel_spmd

FP8 = mybir.dt.float8e4
F32 = mybir.dt.float32
NP_FP8 = ml_dtypes.float8_e4m3  # TRN fp8e4 (max 240, has inf)

N_CORES = 8
P = 128


# ---------------------------------------------------------------------------
# Workaround: this walrus build rejects >1 sem-wait on the Tile tail Drain
# ("Too many sync wait commands"). Split the waits across single-wait drains.
def _drain_and_barrier(self, tick_clock, wait_clock):
    drain_inst = self.nc.sync.drain()
    wait_clock.add_sem_waits(
        drain_inst.ins, ScopedClock({None: tick_clock.global_clock})
    )
    w = list(drain_inst.ins.sync_info.on_wait)
    if len(w) > 1:
        drain_inst.ins.sync_info = mybir.SyncInfo(on_wait=[w[0]], on_update=[])
        for extra in w[1:]:
            d2 = self.nc.sync.drain()
            d2.ins.sync_info = mybir.SyncInfo(on_wait=[extra], on_update=[])
    self.nc.all_engine_barrier()
    assert self.sems is not None
    popped = self.nc._tile_sem_poison_stack.pop()
    assert popped is self._sem_poison
    self.nc.clear_and_free_semaphores(list(self.sems.allocated().values()))
    self.nc.all_engine_barrier()


TileContext._drain_and_barrier = _drain_and_barrier


def split_sync_waits(nc, max_waits=1):
    """Hoist extra sem-waits onto standalone EventSemaphore carriers.

    This walrus build's setupSyncWait rejects instructions carrying more
    than one sem-wait ("Too many sync wait commands"), so any instruction
    with N>1 waits becomes N-1 single-wait EventSemaphore instructions on
    the same engine followed by the original instruction with one wait.
    """
    n_new = 0
    for f in nc.m.functions:
        for blk in f.blocks:
            out = []
            changed = False
            for inst in blk.instructions:
                si = inst.sync_info
                w = list(si.on_wait) if si is not None else []
                if len(w) > max_waits:
                    upd = list(inst.sync_info.on_update)
                    for wi in w[max_waits:]:
                        es = mybir.InstEventSemaphore(
                            name=f"hoistw-{n_new}", ins=[], outs=[]
                        )
                        n_new += 1
                        es.engine = inst.engine
                        es.sync_info = mybir.SyncInfo(on_wait=[wi], on_update=[])
                        out.append(es)
                    inst.sync_info = mybir.SyncInfo(
                        on_wait=w[:max_waits], on_update=upd
                    )
                    changed = True
                out.append(inst)
            if changed:
                blk.instructions = out
    return nc
# ---------------------------------------------------------------------------


USE_DOUBLE_ROW = True  # fp8 DoubleRow: ~1.5x matmul rate, ~1e-4 accum noise


def build(K, M, N, MF=512, use_dr=None):
    """One-core program: out_t[N, M] = requantized (x @ w.T + b) transposed.

    DRAM inputs:
      xt     [K, M]  f32    x shard, transposed (k-major)
      wt     [N//128, 128, K//128, 128]  fp8   halved weight, tiled
                     wt[nt, p, j, n] = fp8(w[nt*128+n, j*128+p] / 2)
      bias2  [N]     f32    bias / (2*s_out)
      inv2si, alpha, two_os  [1, 1] f32:
                     1/(2*s_in),  2*s_in*s_w/s_out,  2*s_out
    Output:
      out_t  [N, M]  f32
    """
    if use_dr is None:
        use_dr = USE_DOUBLE_ROW
    KS = K // P          # k subtiles of 128
    JP = KS // 2         # DoubleRow pairs
    NT = N // P          # n tiles
    MB = M // MF         # m blocks
    AF = mybir.ActivationFunctionType
    OP = mybir.AluOpType

    nc = bass.Bass()
    xt = nc.dram_tensor("xt", [K, M], F32, kind="ExternalInput")
    wt = nc.dram_tensor("wt", [NT, P, KS, P], FP8, kind="ExternalInput")
    bias2_d = nc.dram_tensor("bias2", [N], F32, kind="ExternalInput")
    inv2si_d = nc.dram_tensor("inv2si", [1, 1], F32, kind="ExternalInput")
    alpha_d = nc.dram_tensor("alpha", [1, 1], F32, kind="ExternalInput")
    two_os_d = nc.dram_tensor("two_os", [1, 1], F32, kind="ExternalInput")
    out_t = nc.dram_tensor("out_t", [N, M], F32, kind="ExternalOutput")

    with TileContext(nc) as tc:
        with (
            tc.tile_pool(name="consts", bufs=1) as consts,
            tc.tile_pool(name="wres", bufs=1) as wres,
            tc.tile_pool(name="qx", bufs=2) as qxp,
            tc.tile_pool(name="xf", bufs=6) as xfp,
            tc.tile_pool(name="psum", bufs=8, space="PSUM") as psp,
            tc.tile_pool(name="epi", bufs=3) as epi,
            tc.tile_pool(name="q8", bufs=3) as q8p,
            tc.tile_pool(name="yout", bufs=4) as yp,
        ):
            # ---- per-partition broadcast of the scalars ----
            inv2si = consts.tile([P, 1], F32)
            alpha = consts.tile([P, 1], F32)
            two_os = consts.tile([P, 1], F32)
            nc.gpsimd.dma_start(inv2si[:], inv2si_d[0:1, 0:1].to_broadcast((P, 1)))
            nc.gpsimd.dma_start(alpha[:], alpha_d[0:1, 0:1].to_broadcast((P, 1)))
            nc.gpsimd.dma_start(two_os[:], two_os_d[0:1, 0:1].to_broadcast((P, 1)))

            # bias2[p, nt] = bias[nt*128+p] / (2*os)
            bias2 = consts.tile([P, NT], F32)
            nc.gpsimd.dma_start(bias2[:], bias2_d.rearrange("(nt p) -> p nt", p=P))

            # quantize chunk j of block mb: qx[p, j, m] = fp8(x/(2si)).
            # No explicit clamp: |x/(2si)| <= 224*(1+2^-23) by construction
            # of input_scale, and fp8 RNE rounds anything < 232 down to 224,
            # so the mult+cast is exact clamp semantics. Single DVE op keeps
            # ScalarE as a dedicated psum evictor.
            def emit_quant(mb, j, qx):
                xf = xfp.tile([P, MF], F32, tag="xf", name="xf")
                nc.sync.dma_start(
                    xf[:], xt[j * P : (j + 1) * P, mb * MF : (mb + 1) * MF]
                )
                nc.vector.tensor_scalar(
                    qx[:, j, :], xf[:], inv2si[:, 0:1], None, OP.mult
                )

            # resident halved weight, one tile per nt for per-tile dep
            # tracking; loads are interleaved with the first block's x
            # quantize so the PE can start as soon as w[0] + 2 chunks land
            w_tiles = []

            def emit_wload(nt):
                w_nt = wres.tile([P, KS, P], FP8, tag=f"w{nt}", name=f"w{nt}")
                nc.sync.dma_start(w_nt[:], wt[nt, :, :, :])
                w_tiles.append(w_nt)

            # x chunks get queue priority (the PE is gated on the full qx0);
            # only the warm-up groups' weights are hoisted between early
            # chunks, the rest trail (consumed at ~1/3 the delivery rate)
            NW = min(8, NT)
            qx_tiles = {0: qxp.tile([P, KS, MF], FP8, tag="qx", name="qx0")}
            emit_wload(0)
            emit_wload(1)
            emit_quant(0, 0, qx_tiles[0])
            emit_quant(0, 1, qx_tiles[0])
            for j in range(2, KS):
                emit_quant(0, j, qx_tiles[0])
                if j < NW:
                    emit_wload(j)
            for nt in range(NW, NT):
                emit_wload(nt)

            def emit_mms(ps, nt, qx):
                if use_dr:
                    for jj in range(JP):
                        nc.tensor.matmul(
                            ps[:],
                            w_tiles[nt][:, 2 * jj : 2 * jj + 2, :],
                            qx[:, 2 * jj : 2 * jj + 2, :],
                            start=(jj == 0),
                            stop=(jj == JP - 1),
                            perf_mode=mybir.MatmulPerfMode.DoubleRow,
                        )
                else:
                    for j in range(KS):
                        nc.tensor.matmul(
                            ps[:],
                            w_tiles[nt][:, j, :],
                            qx[:, j, :],
                            start=(j == 0),
                            stop=(j == KS - 1),
                        )

            # epilogue: t = ps*alpha + bias/(2os); q8 = fp8(clamp t);
            # y = q8 * 2os
            def emit_epilogue(ps, nt, mb):
                t = epi.tile([P, MF], F32, tag="t", name="t")
                nc.scalar.activation(
                    t[:], ps[:], AF.Identity,
                    bias=bias2[:, nt : nt + 1], scale=alpha[:, 0:1],
                )
                q8 = q8p.tile([P, MF], FP8, tag="q8", name="q8")
                nc.vector.tensor_scalar(
                    q8[:], t[:], -224.0, 224.0, OP.max, OP.min
                )
                y = yp.tile([P, MF], F32, tag="y", name="y")
                nc.vector.tensor_scalar_mul(y[:], q8[:], two_os[:, 0:1])
                nc.gpsimd.dma_start(
                    out_t[nt * P : (nt + 1) * P, mb * MF : (mb + 1) * MF],
                    y[:],
                )

            # ---- main loop over m blocks ----
            for mb in range(MB):
                qx = qx_tiles[mb]
                if mb + 1 < MB:
                    qx_tiles[mb + 1] = qxp.tile(
                        [P, KS, MF], FP8, tag="qx", name=f"qx{mb+1}"
                    )

                if mb == 0 and use_dr:
                    # warm-up: first NW groups accumulate k-outer across NW
                    # psum banks, so the PE issues NW matmuls per arriving
                    # chunk pair instead of idling for the full qx0
                    ps_warm = [
                        psp.tile([P, MF], F32, tag="ps", name=f"psw{g}")
                        for g in range(NW)
                    ]
                    for jj in range(JP):
                        for g in range(NW):
                            nc.tensor.matmul(
                                ps_warm[g][:],
                                w_tiles[g][:, 2 * jj : 2 * jj + 2, :],
                                qx[:, 2 * jj : 2 * jj + 2, :],
                                start=(jj == 0),
                                stop=(jj == JP - 1),
                                perf_mode=mybir.MatmulPerfMode.DoubleRow,
                            )
                    for g in range(NW):
                        emit_epilogue(ps_warm[g], g, mb)
                    nt_range = list(range(NW, NT))
                else:
                    nt_range = list(range(NT))

                for idx, nt in enumerate(nt_range):
                    ps = psp.tile([P, MF], F32, tag="ps", name="ps")
                    emit_mms(ps, nt, qx)
                    emit_epilogue(ps, nt, mb)
                    # interleave next block's quantize so its DMA/DVE work
                    # lands well ahead of this block's end (keeps the PE warm)
                    if mb + 1 < MB:
                        lo = idx * KS // len(nt_range)
                        hi = (idx + 1) * KS // len(nt_range)
                        for jq in range(lo, hi):
                            emit_quant(mb + 1, jq, qx_tiles[mb + 1])
    return split_sync_waits(nc)


def prep_weight(weight):
    """[N, K] f32 (e4m3fn-grid values) -> [NT, 128, KS, 128] TRN-fp8 of w/2."""
    N, K = weight.shape
    wq = (weight.astype(np.float32) * 0.5).astype(NP_FP8)
    # [nt, n, j, p] -> [nt, p, j, n]
    return np.ascontiguousarray(
        wq.reshape(N // P, P, K // P, P).transpose(0, 3, 2, 1)
    )


def prep_scalars(weight_scale, bias, input_scale, output_scale):
    si = float(np.asarray(input_scale, np.float64))
    sw = float(np.asarray(weight_scale, np.float64))
    os_ = float(np.asarray(output_scale, np.float64))
    inv2si = np.array([[1.0 / (2.0 * si)]], np.float32)
    alpha = np.array([[2.0 * si * sw / os_]], np.float32)
    two_os = np.array([[2.0 * os_]], np.float32)
    bias2 = (bias.astype(np.float64) / (2.0 * os_)).astype(np.float32)
    return inv2si, alpha, two_os, np.ascontiguousarray(bias2)


def kernel(x, weight, weight_scale, bias, input_scale, output_scale):
    x = np.asarray(x, np.float32)
    weight = np.asarray(weight, np.float32)
    bias = np.asarray(bias, np.float32)
    B, S, K = x.shape
    N = weight.shape[0]
    M_total = B * S
    M = M_total // N_CORES

    nc = build(K, M, N)

    xt_full = np.ascontiguousarray(x.reshape(M_total, K).T)  # [K, M_total] f32
    wt = prep_weight(weight)
    inv2si, alpha, two_os, bias2 = prep_scalars(
        weight_scale, bias, input_scale, output_scale
    )

    in_maps = []
    for c in range(N_CORES):
        in_maps.append({
            "xt": np.ascontiguousarray(xt_full[:, c * M : (c + 1) * M]),
            "wt": wt,
            "bias2": bias2,
            "inv2si": inv2si,
            "alpha": alpha,
            "two_os": two_os,
        })

    res = None
    last_exc = None
    for attempt in range(3):
        try:
            res = run_bass_kernel_spmd(nc, in_maps, core_ids=list(range(N_CORES)))
            break
        except Exception as e:  # transient NRT/device errors: retry
            last_exc = e
    if res is None:
        raise last_exc
    global LAST_RESULT
    LAST_RESULT = res

    out = np.empty((M_total, N), np.float32)
    for c in range(N_CORES):
        out[c * M : (c + 1) * M, :] = res.results[c]["out_t"].T
    return out.reshape(B, S, N)


# revision 29
# speedup vs baseline: 1.0871x; 1.0052x over previous
"""FP8StaticLinear Trainium2 kernel.

out = requant_fp8(qdq_fp8(x, s_in) @ (w * s_w).T + bias, s_out)

Sharding: data-parallel over tokens (B*S=16384 -> 2048/core on 8 cores).
Device math: fp8e4 DoubleRow matmuls on the PE array. Both operands are
halved on entry so the OCP-e4m3fn grid (max 448) maps onto TRN fp8e4
(max 240) exactly; scales are folded back in the epilogue.
"""

import numpy as np
import ml_dtypes

import concourse.bass as bass
import concourse.mybir as mybir
from concourse.tile import TileContext
from concourse.vector_clock import ScopedClock
from concourse.bass_utils import run_bass_kernel_spmd

FP8 = mybir.dt.float8e4
F32 = mybir.dt.float32
NP_FP8 = ml_dtypes.float8_e4m3  # TRN fp8e4 (max 240, has inf)

N_CORES = 8
P = 128


# ---------------------------------------------------------------------------
# Workaround: this walrus build rejects >1 sem-wait on the Tile tail Drain
# ("Too many sync wait commands"). Split the waits across single-wait drains.
def _drain_and_barrier(self, tick_clock, wait_clock):
    drain_inst = self.nc.sync.drain()
    wait_clock.add_sem_waits(
        drain_inst.ins, ScopedClock({None: tick_clock.global_clock})
    )
    w = list(drain_inst.ins.sync_info.on_wait)
    if len(w) > 1:
        drain_inst.ins.sync_info = mybir.SyncInfo(on_wait=[w[0]], on_update=[])
        for extra in w[1:]:
            d2 = self.nc.sync.drain()
            d2.ins.sync_info = mybir.SyncInfo(on_wait=[extra], on_update=[])
    self.nc.all_engine_barrier()
    assert self.sems is not None
    popped = self.nc._tile_sem_poison_stack.pop()
    assert popped is self._sem_poison
    self.nc.clear_and_free_semaphores(list(self.sems.allocated().values()))
    self.nc.all_engine_barrier()


TileContext._drain_and_barrier = _drain_and_barrier


def split_sync_waits(nc, max_waits=1):
    """Hoist extra sem-waits onto standalone EventSemaphore carriers.

    This walrus build's setupSyncWait rejects instructions carrying more
    than one sem-wait ("Too many sync wait commands"), so any instruction
    with N>1 waits becomes N-1 single-wait EventSemaphore instructions on
    the same engine followed by the original instruction with one wait.
    """
    n_new = 0
    for f in nc.m.functions:
        for blk in f.blocks:
            out = []
            changed = False
            for inst in blk.instructions:
                si = inst.sync_info
                w = list(si.on_wait) if si is not None else []
                if len(w) > max_waits:
                    upd = list(inst.sync_info.on_update)
                    for wi in w[max_waits:]:
                        es = mybir.InstEventSemaphore(
                            name=f"hoistw-{n_new}", ins=[], outs=[]
                        )
                        n_new += 1
                        es.engine = inst.engine
                        es.sync_info = mybir.SyncInfo(on_wait=[wi], on_update=[])
                        out.append(es)
                    inst.sync_info = mybir.SyncInfo(
                        on_wait=w[:max_waits], on_update=upd
                    )
                    changed = True
                out.append(inst)
            if changed:
                blk.instructions = out
    return nc
# ---------------------------------------------------------------------------


USE_DOUBLE_ROW = True  # fp8 DoubleRow: ~1.5x matmul rate, ~1e-4 accum noise


def build(K, M, N, MF=512, use_dr=None):
    """One-core program: out_t[N, M] = requantized (x @ w.T + b) transposed.

    DRAM inputs:
      xt     [K, M]  f32    x shard, transposed (k-major)
      wt     [N//128, 128, K//128, 128]  fp8   halved weight, tiled
                     wt[nt, p, j, n] = fp8(w[nt*128+n, j*128+p] / 2)
      bias2  [N]     f32    bias / (2*s_out)
      inv2si, alpha, two_os  [1, 1] f32:
                     1/(2*s_in),  2*s_in*s_w/s_out,  2*s_out
    Output:
      out_t  [N, M]  f32
    """
    if use_dr is None:
        use_dr = USE_DOUBLE_ROW
    KS = K // P          # k subtiles of 128
    JP = KS // 2         # DoubleRow pairs
    NT = N // P          # n tiles
    MB = M // MF         # m blocks
    AF = mybir.ActivationFunctionType
    OP = mybir.AluOpType

    nc = bass.Bass()
    xt = nc.dram_tensor("xt", [K, M], F32, kind="ExternalInput")
    wt = nc.dram_tensor("wt", [NT, P, KS, P], FP8, kind="ExternalInput")
    bias2_d = nc.dram_tensor("bias2", [N], F32, kind="ExternalInput")
    inv2si_d = nc.dram_tensor("inv2si", [1, 1], F32, kind="ExternalInput")
    alpha_d = nc.dram_tensor("alpha", [1, 1], F32, kind="ExternalInput")
    two_os_d = nc.dram_tensor("two_os", [1, 1], F32, kind="ExternalInput")
    out_t = nc.dram_tensor("out_t", [N, M], F32, kind="ExternalOutput")

    with TileContext(nc) as tc:
        with (
            tc.tile_pool(name="consts", bufs=1) as consts,
            tc.tile_pool(name="wres", bufs=1) as wres,
            tc.tile_pool(name="qx", bufs=2) as qxp,
            tc.tile_pool(name="xf", bufs=6) as xfp,
            tc.tile_pool(name="psum", bufs=8, space="PSUM") as psp,
            tc.tile_pool(name="epi", bufs=3) as epi,
            tc.tile_pool(name="q8", bufs=3) as q8p,
            tc.tile_pool(name="yout", bufs=4) as yp,
        ):
            # ---- per-partition broadcast of the scalars ----
            inv2si = consts.tile([P, 1], F32)
            alpha = consts.tile([P, 1], F32)
            two_os = consts.tile([P, 1], F32)
            nc.gpsimd.dma_start(inv2si[:], inv2si_d[0:1, 0:1].to_broadcast((P, 1)))
            nc.gpsimd.dma_start(alpha[:], alpha_d[0:1, 0:1].to_broadcast((P, 1)))
            nc.gpsimd.dma_start(two_os[:], two_os_d[0:1, 0:1].to_broadcast((P, 1)))

            # bias2[p, nt] = bias[nt*128+p] / (2*os)
            bias2 = consts.tile([P, NT], F32)
            nc.gpsimd.dma_start(bias2[:], bias2_d.rearrange("(nt p) -> p nt", p=P))

            # quantize chunk j of block mb: qx[p, j, m] = fp8(x/(2si)).
            # No explicit clamp: |x/(2si)| <= 224*(1+2^-23) by construction
            # of input_scale, and fp8 RNE rounds anything < 232 down to 224,
            # so the mult+cast is exact clamp semantics. Single DVE op keeps
            # ScalarE as a dedicated psum evictor.
            def emit_quant(mb, j, qx):
                xf = xfp.tile([P, MF], F32, tag="xf", name="xf")
                nc.sync.dma_start(
                    xf[:], xt[j * P : (j + 1) * P, mb * MF : (mb + 1) * MF]
                )
                nc.vector.tensor_scalar(
                    qx[:, j, :], xf[:], inv2si[:, 0:1], None, OP.mult
                )

            # resident halved weight, one tile per nt for per-tile dep
            # tracking; loads are interleaved with the first block's x
            # quantize so the PE can start as soon as w[0] + 2 chunks land
            w_tiles = []

            def emit_wload(nt):
                w_nt = wres.tile([P, KS, P], FP8, tag=f"w{nt}", name=f"w{nt}")
                h = KS // 2
                nc.sync.dma_start(w_nt[:, :h, :], wt[nt, :, :h, :])
                nc.sync.dma_start(w_nt[:, h:, :], wt[nt, :, h:, :])
                w_tiles.append(w_nt)

            # x chunks get queue priority (the PE is gated on the full qx0);
            # only the warm-up groups' weights are hoisted between early
            # chunks, the rest trail (consumed at ~1/3 the delivery rate)
            NW = min(8, NT)
            qx_tiles = {0: qxp.tile([P, KS, MF], FP8, tag="qx", name="qx0")}
            emit_wload(0)
            emit_wload(1)
            emit_quant(0, 0, qx_tiles[0])
            emit_quant(0, 1, qx_tiles[0])
            for j in range(2, KS):
                emit_quant(0, j, qx_tiles[0])
                if j < NW:
                    emit_wload(j)
            for nt in range(NW, NT):
                emit_wload(nt)

            def emit_mms(ps, nt, qx):
                if use_dr:
                    for jj in range(JP):
                        nc.tensor.matmul(
                            ps[:],
                            w_tiles[nt][:, 2 * jj : 2 * jj + 2, :],
                            qx[:, 2 * jj : 2 * jj + 2, :],
                            start=(jj == 0),
                            stop=(jj == JP - 1),
                            perf_mode=mybir.MatmulPerfMode.DoubleRow,
                        )
                else:
                    for j in range(KS):
                        nc.tensor.matmul(
                            ps[:],
                            w_tiles[nt][:, j, :],
                            qx[:, j, :],
                            start=(j == 0),
                            stop=(j == KS - 1),
                        )

            # epilogue: t = ps*alpha + bias/(2os); q8 = fp8(clamp t);
            # y = q8 * 2os
            def emit_epilogue(ps, nt, mb):
                t = epi.tile([P, MF], F32, tag="t", name="t")
                nc.scalar.activation(
                    t[:], ps[:], AF.Identity,
                    bias=bias2[:, nt : nt + 1], scale=alpha[:, 0:1],
                )
                q8 = q8p.tile([P, MF], FP8, tag="q8", name="q8")
                nc.vector.tensor_scalar(
                    q8[:], t[:], -224.0, 224.0, OP.max, OP.min
                )
                y = yp.tile([P, MF], F32, tag="y", name="y")
                nc.vector.tensor_scalar_mul(y[:], q8[:], two_os[:, 0:1])
                nc.gpsimd.dma_start(
                    out_t[nt * P : (nt + 1) * P, mb * MF : (mb + 1) * MF],
                    y[:],
                )

            # ---- main loop over m blocks ----
            for mb in range(MB):
                qx = qx_tiles[mb]
                if mb + 1 < MB:
                    qx_tiles[mb + 1] = qxp.tile(
                        [P, KS, MF], FP8, tag="qx", name=f"qx{mb+1}"
                    )

                if mb == 0 and use_dr:
                    # warm-up: first NW groups accumulate k-outer across NW
                    # psum banks, so the PE issues NW matmuls per arriving
                    # chunk pair instead of idling for the full qx0
                    ps_warm = [
                        psp.tile([P, MF], F32, tag="ps", name=f"psw{g}")
                        for g in range(NW)
                    ]
                    for jj in range(JP):
                        for g in range(NW):
                            nc.tensor.matmul(
                                ps_warm[g][:],
                                w_tiles[g][:, 2 * jj : 2 * jj + 2, :],
                                qx[:, 2 * jj : 2 * jj + 2, :],
                                start=(jj == 0),
                                stop=(jj == JP - 1),
                                perf_mode=mybir.MatmulPerfMode.DoubleRow,
                            )
                    for g in range(NW):
                        emit_epilogue(ps_warm[g], g, mb)
                    nt_range = list(range(NW, NT))
                else:
                    nt_range = list(range(NT))

                for idx, nt in enumerate(nt_range):
                    ps = psp.tile([P, MF], F32, tag="ps", name="ps")
                    emit_mms(ps, nt, qx)
                    emit_epilogue(ps, nt, mb)
                    # interleave next block's quantize so its DMA/DVE work
                    # lands well ahead of this block's end (keeps the PE warm)
                    if mb + 1 < MB:
                        lo = idx * KS // len(nt_range)
                        hi = (idx + 1) * KS // len(nt_range)
                        for jq in range(lo, hi):
                            emit_quant(mb + 1, jq, qx_tiles[mb + 1])
    return split_sync_waits(nc)


def prep_weight(weight):
    """[N, K] f32 (e4m3fn-grid values) -> [NT, 128, KS, 128] TRN-fp8 of w/2."""
    N, K = weight.shape
    wq = (weight.astype(np.float32) * 0.5).astype(NP_FP8)
    # [nt, n, j, p] -> [nt, p, j, n]
    return np.ascontiguousarray(
        wq.reshape(N // P, P, K // P, P).transpose(0, 3, 2, 1)
    )


def prep_scalars(weight_scale, bias, input_scale, output_scale):
    si = float(np.asarray(input_scale, np.float64))
    sw = float(np.asarray(weight_scale, np.float64))
    os_ = float(np.asarray(output_scale, np.float64))
    inv2si = np.array([[1.0 / (2.0 * si)]], np.float32)
    alpha = np.array([[2.0 * si * sw / os_]], np.float32)
    two_os = np.array([[2.0 * os_]], np.float32)
    bias2 = (bias.astype(np.float64) / (2.0 * os_)).astype(np.float32)
    return inv2si, alpha, two_os, np.ascontiguousarray(bias2)


def kernel(x, weight, weight_scale, bias, input_scale, output_scale):
    x = np.asarray(x, np.float32)
    weight = np.asarray(weight, np.float32)
    bias = np.asarray(bias, np.float32)
    B, S, K = x.shape
    N = weight.shape[0]
    M_total = B * S
    M = M_total // N_CORES

    nc = build(K, M, N)

    xt_full = np.ascontiguousarray(x.reshape(M_total, K).T)  # [K, M_total] f32
    wt = prep_weight(weight)
    inv2si, alpha, two_os, bias2 = prep_scalars(
        weight_scale, bias, input_scale, output_scale
    )

    in_maps = []
    for c in range(N_CORES):
        in_maps.append({
            "xt": np.ascontiguousarray(xt_full[:, c * M : (c + 1) * M]),
            "wt": wt,
            "bias2": bias2,
            "inv2si": inv2si,
            "alpha": alpha,
            "two_os": two_os,
        })

    res = None
    last_exc = None
    for attempt in range(3):
        try:
            res = run_bass_kernel_spmd(nc, in_maps, core_ids=list(range(N_CORES)))
            break
        except Exception as e:  # transient NRT/device errors: retry
            last_exc = e
    if res is None:
        raise last_exc
    global LAST_RESULT
    LAST_RESULT = res

    out = np.empty((M_total, N), np.float32)
    for c in range(N_CORES):
        out[c * M : (c + 1) * M, :] = res.results[c]["out_t"].T
    return out.reshape(B, S, N)


# revision 30
# speedup vs baseline: 1.0905x; 1.0031x over previous
"""FP8StaticLinear Trainium2 kernel.

out = requant_fp8(qdq_fp8(x, s_in) @ (w * s_w).T + bias, s_out)

Sharding: data-parallel over tokens (B*S=16384 -> 2048/core on 8 cores).
Device math: fp8e4 DoubleRow matmuls on the PE array. Both operands are
halved on entry so the OCP-e4m3fn grid (max 448) maps onto TRN fp8e4
(max 240) exactly; scales are folded back in the epilogue.
"""

import numpy as np
import ml_dtypes

import concourse.bass as bass
import concourse.mybir as mybir
from concourse.tile import TileContext
from concourse.vector_clock import ScopedClock
from concourse.bass_utils import run_bass_kernel_spmd

FP8 = mybir.dt.float8e4
F32 = mybir.dt.float32
NP_FP8 = ml_dtypes.float8_e4m3  # TRN fp8e4 (max 240, has inf)

N_CORES = 8
P = 128


# ---------------------------------------------------------------------------
# Workaround: this walrus build rejects >1 sem-wait on the Tile tail Drain
# ("Too many sync wait commands"). Split the waits across single-wait drains.
def _drain_and_barrier(self, tick_clock, wait_clock):
    drain_inst = self.nc.sync.drain()
    wait_clock.add_sem_waits(
        drain_inst.ins, ScopedClock({None: tick_clock.global_clock})
    )
    w = list(drain_inst.ins.sync_info.on_wait)
    if len(w) > 1:
        drain_inst.ins.sync_info = mybir.SyncInfo(on_wait=[w[0]], on_update=[])
        for extra in w[1:]:
            d2 = self.nc.sync.drain()
            d2.ins.sync_info = mybir.SyncInfo(on_wait=[extra], on_update=[])
    self.nc.all_engine_barrier()
    assert self.sems is not None
    popped = self.nc._tile_sem_poison_stack.pop()
    assert popped is self._sem_poison
    self.nc.clear_and_free_semaphores(list(self.sems.allocated().values()))
    self.nc.all_engine_barrier()


TileContext._drain_and_barrier = _drain_and_barrier


def split_sync_waits(nc, max_waits=1):
    """Hoist extra sem-waits onto standalone EventSemaphore carriers.

    This walrus build's setupSyncWait rejects instructions carrying more
    than one sem-wait ("Too many sync wait commands"), so any instruction
    with N>1 waits becomes N-1 single-wait EventSemaphore instructions on
    the same engine followed by the original instruction with one wait.
    """
    n_new = 0
    for f in nc.m.functions:
        for blk in f.blocks:
            out = []
            changed = False
            for inst in blk.instructions:
                si = inst.sync_info
                w = list(si.on_wait) if si is not None else []
                if len(w) > max_waits:
                    upd = list(inst.sync_info.on_update)
                    for wi in w[max_waits:]:
                        es = mybir.InstEventSemaphore(
                            name=f"hoistw-{n_new}", ins=[], outs=[]
                        )
                        n_new += 1
                        es.engine = inst.engine
                        es.sync_info = mybir.SyncInfo(on_wait=[wi], on_update=[])
                        out.append(es)
                    inst.sync_info = mybir.SyncInfo(
                        on_wait=w[:max_waits], on_update=upd
                    )
                    changed = True
                out.append(inst)
            if changed:
                blk.instructions = out
    return nc
# ---------------------------------------------------------------------------


USE_DOUBLE_ROW = True  # fp8 DoubleRow: ~1.5x matmul rate, ~1e-4 accum noise


def build(K, M, N, MF=512, use_dr=None):
    """One-core program: out_t[N, M] = requantized (x @ w.T + b) transposed.

    DRAM inputs:
      xt     [K, M]  f32    x shard, transposed (k-major)
      wt     [N//128, 128, K//128, 128]  fp8   halved weight, tiled
                     wt[nt, p, j, n] = fp8(w[nt*128+n, j*128+p] / 2)
      bias2  [N]     f32    bias / (2*s_out)
      inv2si, alpha, two_os  [1, 1] f32:
                     1/(2*s_in),  2*s_in*s_w/s_out,  2*s_out
    Output:
      out_t  [N, M]  f32
    """
    if use_dr is None:
        use_dr = USE_DOUBLE_ROW
    KS = K // P          # k subtiles of 128
    JP = KS // 2         # DoubleRow pairs
    NT = N // P          # n tiles
    MB = M // MF         # m blocks
    AF = mybir.ActivationFunctionType
    OP = mybir.AluOpType

    nc = bass.Bass()
    xt = nc.dram_tensor("xt", [K, M], F32, kind="ExternalInput")
    wt = nc.dram_tensor("wt", [NT, P, KS, P], FP8, kind="ExternalInput")
    bias2_d = nc.dram_tensor("bias2", [N], F32, kind="ExternalInput")
    inv2si_d = nc.dram_tensor("inv2si", [1, 1], F32, kind="ExternalInput")
    alpha_d = nc.dram_tensor("alpha", [1, 1], F32, kind="ExternalInput")
    two_os_d = nc.dram_tensor("two_os", [1, 1], F32, kind="ExternalInput")
    out_t = nc.dram_tensor("out_t", [N, M], F32, kind="ExternalOutput")

    with TileContext(nc) as tc:
        with (
            tc.tile_pool(name="consts", bufs=1) as consts,
            tc.tile_pool(name="wres", bufs=1) as wres,
            tc.tile_pool(name="qx", bufs=2) as qxp,
            tc.tile_pool(name="xf", bufs=6) as xfp,
            tc.tile_pool(name="psum", bufs=8, space="PSUM") as psp,
            tc.tile_pool(name="epi", bufs=3) as epi,
            tc.tile_pool(name="q8", bufs=3) as q8p,
            tc.tile_pool(name="yout", bufs=4) as yp,
        ):
            # ---- per-partition broadcast of the scalars ----
            inv2si = consts.tile([P, 1], F32)
            alpha = consts.tile([P, 1], F32)
            two_os = consts.tile([P, 1], F32)
            nc.gpsimd.dma_start(inv2si[:], inv2si_d[0:1, 0:1].to_broadcast((P, 1)))
            nc.gpsimd.dma_start(alpha[:], alpha_d[0:1, 0:1].to_broadcast((P, 1)))
            nc.gpsimd.dma_start(two_os[:], two_os_d[0:1, 0:1].to_broadcast((P, 1)))

            # bias2[p, nt] = bias[nt*128+p] / (2*os)
            bias2 = consts.tile([P, NT], F32)
            nc.gpsimd.dma_start(bias2[:], bias2_d.rearrange("(nt p) -> p nt", p=P))

            # quantize chunk j of block mb: qx[p, j, m] = fp8(x/(2si)).
            # No explicit clamp: |x/(2si)| <= 224*(1+2^-23) by construction
            # of input_scale, and fp8 RNE rounds anything < 232 down to 224,
            # so the mult+cast is exact clamp semantics. Single DVE op keeps
            # ScalarE as a dedicated psum evictor.
            def emit_quant(mb, j, qx):
                xf = xfp.tile([P, MF], F32, tag="xf", name="xf")
                nc.sync.dma_start(
                    xf[:], xt[j * P : (j + 1) * P, mb * MF : (mb + 1) * MF]
                )
                nc.vector.tensor_scalar(
                    qx[:, j, :], xf[:], inv2si[:, 0:1], None, OP.mult
                )

            # resident halved weight, one tile per nt for per-tile dep
            # tracking; loads are interleaved with the first block's x
            # quantize so the PE can start as soon as w[0] + 2 chunks land
            w_tiles = []

            def emit_wload(nt):
                w_nt = wres.tile([P, KS, P], FP8, tag=f"w{nt}", name=f"w{nt}")
                h = KS // 2
                nc.sync.dma_start(w_nt[:, :h, :], wt[nt, :, :h, :])
                nc.sync.dma_start(w_nt[:, h:, :], wt[nt, :, h:, :])
                w_tiles.append(w_nt)

            # x chunks get queue priority (the PE is gated on the full qx0);
            # only the warm-up groups' weights are hoisted between early
            # chunks, the rest trail (consumed at ~1/3 the delivery rate)
            NW = min(8, NT)
            qx_tiles = {0: qxp.tile([P, KS, MF], FP8, tag="qx", name="qx0")}
            emit_wload(0)
            emit_wload(1)
            emit_quant(0, 0, qx_tiles[0])
            emit_quant(0, 1, qx_tiles[0])
            for j in range(2, KS):
                emit_quant(0, j, qx_tiles[0])
                if j < NW:
                    emit_wload(j)
            for nt in range(NW, NT):
                emit_wload(nt)

            def emit_mms(ps, nt, qx):
                if use_dr:
                    for jj in range(JP):
                        nc.tensor.matmul(
                            ps[:],
                            w_tiles[nt][:, 2 * jj : 2 * jj + 2, :],
                            qx[:, 2 * jj : 2 * jj + 2, :],
                            start=(jj == 0),
                            stop=(jj == JP - 1),
                            perf_mode=mybir.MatmulPerfMode.DoubleRow,
                        )
                else:
                    for j in range(KS):
                        nc.tensor.matmul(
                            ps[:],
                            w_tiles[nt][:, j, :],
                            qx[:, j, :],
                            start=(j == 0),
                            stop=(j == KS - 1),
                        )

            # epilogue: t = ps*alpha + bias/(2os); q8 = fp8(clamp t);
            # y = q8 * 2os
            def emit_epilogue(ps, nt, mb):
                t = epi.tile([P, MF], F32, tag="t", name="t")
                nc.scalar.activation(
                    t[:], ps[:], AF.Identity,
                    bias=bias2[:, nt : nt + 1], scale=alpha[:, 0:1],
                )
                q8 = q8p.tile([P, MF], FP8, tag="q8", name="q8")
                nc.vector.tensor_scalar(
                    q8[:], t[:], -224.0, 224.0, OP.max, OP.min
                )
                y = yp.tile([P, MF], F32, tag="y", name="y")
                nc.vector.tensor_scalar_mul(y[:], q8[:], two_os[:, 0:1])
                nc.sync.dma_start(
                    out_t[nt * P : (nt + 1) * P, mb * MF : (mb + 1) * MF],
                    y[:],
                )

            # ---- main loop over m blocks ----
            for mb in range(MB):
                qx = qx_tiles[mb]
                if mb + 1 < MB:
                    qx_tiles[mb + 1] = qxp.tile(
                        [P, KS, MF], FP8, tag="qx", name=f"qx{mb+1}"
                    )

                if mb == 0 and use_dr:
                    # warm-up: first NW groups accumulate k-outer across NW
                    # psum banks, so the PE issues NW matmuls per arriving
                    # chunk pair instead of idling for the full qx0
                    ps_warm = [
                        psp.tile([P, MF], F32, tag="ps", name=f"psw{g}")
                        for g in range(NW)
                    ]
                    for jj in range(JP):
                        for g in range(NW):
                            nc.tensor.matmul(
                                ps_warm[g][:],
                                w_tiles[g][:, 2 * jj : 2 * jj + 2, :],
                                qx[:, 2 * jj : 2 * jj + 2, :],
                                start=(jj == 0),
                                stop=(jj == JP - 1),
                                perf_mode=mybir.MatmulPerfMode.DoubleRow,
                            )
                    for g in range(NW):
                        emit_epilogue(ps_warm[g], g, mb)
                    nt_range = list(range(NW, NT))
                else:
                    nt_range = list(range(NT))

                for idx, nt in enumerate(nt_range):
                    ps = psp.tile([P, MF], F32, tag="ps", name="ps")
                    emit_mms(ps, nt, qx)
                    emit_epilogue(ps, nt, mb)
                    # interleave next block's quantize so its DMA/DVE work
                    # lands well ahead of this block's end (keeps the PE warm)
                    if mb + 1 < MB:
                        lo = idx * KS // len(nt_range)
                        hi = (idx + 1) * KS // len(nt_range)
                        for jq in range(lo, hi):
                            emit_quant(mb + 1, jq, qx_tiles[mb + 1])
    return split_sync_waits(nc)


def prep_weight(weight):
    """[N, K] f32 (e4m3fn-grid values) -> [NT, 128, KS, 128] TRN-fp8 of w/2."""
    N, K = weight.shape
    wq = (weight.astype(np.float32) * 0.5).astype(NP_FP8)
    # [nt, n, j, p] -> [nt, p, j, n]
    return np.ascontiguousarray(
        wq.reshape(N // P, P, K // P, P).transpose(0, 3, 2, 1)
    )


def prep_scalars(weight_scale, bias, input_scale, output_scale):
    si = float(np.asarray(input_scale, np.float64))
    sw = float(np.asarray(weight_scale, np.float64))
    os_ = float(np.asarray(output_scale, np.float64))
    inv2si = np.array([[1.0 / (2.0 * si)]], np.float32)
    alpha = np.array([[2.0 * si * sw / os_]], np.float32)
    two_os = np.array([[2.0 * os_]], np.float32)
    bias2 = (bias.astype(np.float64) / (2.0 * os_)).astype(np.float32)
    return inv2si, alpha, two_os, np.ascontiguousarray(bias2)


def kernel(x, weight, weight_scale, bias, input_scale, output_scale):
    x = np.asarray(x, np.float32)
    weight = np.asarray(weight, np.float32)
    bias = np.asarray(bias, np.float32)
    B, S, K = x.shape
    N = weight.shape[0]
    M_total = B * S
    M = M_total // N_CORES

    nc = build(K, M, N)

    xt_full = np.ascontiguousarray(x.reshape(M_total, K).T)  # [K, M_total] f32
    wt = prep_weight(weight)
    inv2si, alpha, two_os, bias2 = prep_scalars(
        weight_scale, bias, input_scale, output_scale
    )

    in_maps = []
    for c in range(N_CORES):
        in_maps.append({
            "xt": np.ascontiguousarray(xt_full[:, c * M : (c + 1) * M]),
            "wt": wt,
            "bias2": bias2,
            "inv2si": inv2si,
            "alpha": alpha,
            "two_os": two_os,
        })

    res = None
    last_exc = None
    for attempt in range(3):
        try:
            res = run_bass_kernel_spmd(nc, in_maps, core_ids=list(range(N_CORES)))
            break
        except Exception as e:  # transient NRT/device errors: retry
            last_exc = e
    if res is None:
        raise last_exc
    global LAST_RESULT
    LAST_RESULT = res

    out = np.empty((M_total, N), np.float32)
    for c in range(N_CORES):
        out[c * M : (c + 1) * M, :] = res.results[c]["out_t"].T
    return out.reshape(B, S, N)
